# revision 1
# baseline (speedup 1.0000x reference)
"""GATv2 (2-layer) Trainium2 Bass kernel, 8-core SPMD.

Strategy (node-partitioned, per sharding hint option 2):
- Core k owns destination nodes [k*6250, (k+1)*6250). Host bins each core's
  edges by dst, sorts by dst, groups them into 64-node destination windows.
- Per layer: each core computes x_l/x_r for its own node slice on the PE
  (lhsT = x^T node tiles, rhs = [w_l^T | w_r^T]), AllGathers the x_l table
  (bf16) into DRAM, and writes its x_r slice to a local DRAM table. Tables
  are stored partition-major (node n at row (n%128)*COLS + n//128) so all
  device copies are contiguous; host computes gather indices accordingly.
- Edge phase: x_l[src] (x_j) and x_r[dst_local] are fetched with dma_gather
  (bf16, 256B rows). Indices must fit int16, so the x_l table is split at
  row 32768; each window's edges are grouped into an A-part (table row <
  32768) and B-part, each padded to whole 128-edge tiles. Per-window tile
  budgets are maxed across cores so all 8 cores run one SPMD program.
- alpha = sum_c leakyrelu(x_j + x_rd, 0.2)*att via tensor_tensor_reduce per
  (tile, head); w = exp(alpha) unnormalized (the segment-softmax max
  subtraction cancels mathematically; alphas are small so exp is safe).
- Segment sum on PE: per tile a host-built one-hot S [128e, 64n] (zero rows
  for pads) is stationary; rhs = [w*x_j | w] (130 cols). PSUM accumulates
  across a window's tiles; dumps land in a node-major SBUF accumulator.
- Finalize: h = acc[:, :128]/(denom+1e-16) + bias (+leaky 0.01 for layer
  1). Layer 2 reuses the identical edge structure. Output = h1 + h2 of the
  owned slice; host reassembles cores.
"""
import os
import numpy as np
import ml_dtypes

BF = ml_dtypes.bfloat16

N = 50000
E = 640000
HID = 128
HEADS = 2
C = 64
ATT_SLOPE = 0.2
OUT_SLOPE = 0.01
NCORES = 8
TILE = 128
WIN = 64
SIM_LEAKY = bool(int(os.environ.get("GAT_SIM_LEAKY", "0")))


def configure(n=50000, e=640000, chunk_tiles=16, split=32768):
    global N, E, NPC, NPC_PAD, WINS, NTN, NP_GLOB, SPLIT, CHUNK_TILES
    N, E = n, e
    NPC = N // NCORES
    NPC_PAD = ((NPC + TILE - 1) // TILE) * TILE
    WINS = NPC_PAD // WIN
    NTN = NPC_PAD // TILE
    NP_GLOB = NPC_PAD * NCORES
    SPLIT = split
    CHUNK_TILES = chunk_tiles


configure()


def _pack_idx16(idx):
    n = len(idx)
    cols = -(-n // 16)
    buf = np.zeros((cols, 16), dtype=np.int16)
    buf.reshape(-1)[:n] = idx.astype(np.int16)
    return np.tile(buf.T, (8, 1))


def _table_row(node_pad_global):
    """Row of a padded-global node in the p-major gather table."""
    k = node_pad_global // NPC_PAD
    loc = node_pad_global % NPC_PAD
    return k * NPC_PAD + (loc % 128) * NTN + loc // 128


def prep_edges(edge_index):
    src = np.asarray(edge_index[0], dtype=np.int64)
    dst = np.asarray(edge_index[1], dtype=np.int64)
    core_of = dst // NPC
    gp_all = (src // NPC) * NPC_PAD + (src % NPC)
    row_all = _table_row(gp_all)

    cores = []
    cnt = np.zeros((NCORES, WINS, 2), dtype=np.int64)
    for k in range(NCORES):
        m = core_of == k
        rj = row_all[m]
        dl = dst[m] - k * NPC
        order = np.argsort(dl, kind="stable")
        rj, dl = rj[order], dl[order]
        w = dl // WIN
        a = rj < SPLIT
        cnt[k, :, 0] = np.bincount(w[a], minlength=WINS)
        cnt[k, :, 1] = np.bincount(w[~a], minlength=WINS)
        cores.append((rj, dl, w, a))

    budget = (-(-cnt // TILE)).max(axis=0)      # [WINS, 2]
    tile_win, tile_part = [], []
    for part in (0, 1):
        for wi in range(WINS):
            tile_win += [wi] * int(budget[wi, part])
            tile_part += [part] * int(budget[wi, part])
    t_tot = len(tile_win)
    t_a = int(budget[:, 0].sum())

    per_core = []
    for k in range(NCORES):
        rj, dl, w, a = cores[k]
        idx_j = np.zeros(t_tot * TILE, dtype=np.int16)
        idx_d = np.zeros(t_tot * TILE, dtype=np.int16)
        s_col = np.full(t_tot * TILE, -1, dtype=np.int64)
        pos = 0
        for part in (0, 1):
            mp = a if part == 0 else ~a
            for wi in range(WINS):
                sel = mp & (w == wi)
                r_sel, d_sel = rj[sel], dl[sel]
                n_real = len(r_sel)
                n_slot = int(budget[wi, part]) * TILE
                idx_j[pos:pos + n_real] = (r_sel - (SPLIT if part else 0)).astype(np.int16)
                # dst-local gather row in the p-major x_r table
                idx_d[pos:pos + n_real] = ((d_sel % 128) * NTN + d_sel // 128).astype(np.int16)
                s_col[pos:pos + n_real] = d_sel - wi * WIN
                pos += n_slot
        assert pos == t_tot * TILE

        s_mat = np.zeros((t_tot * TILE, WIN), dtype=BF)
        real = s_col >= 0
        s_mat[np.nonzero(real)[0], s_col[real]] = 1.0
        s_mat = s_mat.reshape(t_tot, TILE, WIN).transpose(1, 0, 2)

        per_core.append({
            "idxJ": _pack_idx16(idx_j),
            "idxD": _pack_idx16(idx_d),
            "Smat": np.ascontiguousarray(s_mat),
        })

    meta = {"t_tot": t_tot, "t_a": t_a,
            "tile_win": tile_win, "tile_part": tile_part}
    return meta, per_core


def build_bass(meta):
    from concourse import bacc, mybir, tile

    F32, BF16, I16 = mybir.dt.float32, mybir.dt.bfloat16, mybir.dt.int16
    AF = mybir.ActivationFunctionType
    ALU = mybir.AluOpType

    t_tot, t_a = meta["t_tot"], meta["t_a"]
    tile_win, tile_part = meta["tile_win"], meta["tile_part"]
    n_chunks = -(-t_tot // CHUNK_TILES)

    nc = bacc.Bacc("TRN2", target_bir_lowering=False, debug=False,
                   num_devices=NCORES)

    xT_d = nc.dram_tensor("xT", [HID, NPC_PAD], BF16, kind="ExternalInput")
    w1_d = nc.dram_tensor("w1T", [HID, 2 * HID], BF16, kind="ExternalInput")
    w2_d = nc.dram_tensor("w2T", [HID, 2 * HID], BF16, kind="ExternalInput")
    att1_d = nc.dram_tensor("attbc1", [128, HID], BF16, kind="ExternalInput")
    att2_d = nc.dram_tensor("attbc2", [128, HID], BF16, kind="ExternalInput")
    b1_d = nc.dram_tensor("bias1", [128, HID], F32, kind="ExternalInput")
    b2_d = nc.dram_tensor("bias2", [128, HID], F32, kind="ExternalInput")
    id_d = nc.dram_tensor("ident", [128, 128], BF16, kind="ExternalInput")
    idxj_d = nc.dram_tensor("idxJ", [128, t_tot * 8], I16, kind="ExternalInput")
    idxd_d = nc.dram_tensor("idxD", [128, t_tot * 8], I16, kind="ExternalInput")
    smat_d = nc.dram_tensor("Smat", [128, t_tot, WIN], BF16, kind="ExternalInput")
    out_d = nc.dram_tensor("out", [128, NTN, HID], F32, kind="ExternalOutput")

    with tile.TileContext(nc) as tc:
        with (
            tc.tile_pool(name="const", bufs=1) as cpool,
            tc.tile_pool(name="node", bufs=1) as npool,
            tc.tile_pool(name="edge", bufs=2) as epool,
            tc.tile_pool(name="mmps", bufs=2, space="PSUM") as mmps,
            tc.tile_pool(name="wps", bufs=4, space="PSUM") as wps,
            tc.tile_pool(name="dram", bufs=1, space="DRAM") as dpool,
        ):
            w1_t = cpool.tile([HID, 2 * HID], BF16, tag="w1")
            w2_t = cpool.tile([HID, 2 * HID], BF16, tag="w2")
            att1_t = cpool.tile([128, HID], BF16, tag="att1")
            att2_t = cpool.tile([128, HID], BF16, tag="att2")
            b1_t = cpool.tile([128, HID], F32, tag="b1")
            b2_t = cpool.tile([128, HID], F32, tag="b2")
            id_t = cpool.tile([128, 128], BF16, tag="ident")

            for tdst, tsrc in ((w1_t, w1_d), (w2_t, w2_d), (att1_t, att1_d),
                               (att2_t, att2_d), (b1_t, b1_d), (b2_t, b2_d),
                               (id_t, id_d)):
                nc.sync.dma_start(tdst[:], tsrc[:])

            xT_t = npool.tile([HID, NPC_PAD], BF16, tag="xT")
            nc.sync.dma_start(xT_t[:], xT_d[:])

            h1_t = npool.tile([128, NTN, HID], BF16, tag="h1")
            h1T_t = npool.tile([HID, NPC_PAD], BF16, tag="h1T")
            acc_t = npool.tile([128, WINS // 2, HID + 2], F32, tag="acc")

            cin = dpool.tile([128, NPC_PAD], BF16, tag="cin")
            xl_full = dpool.tile([NP_GLOB, HID], BF16, tag="xlfull")
            xr_loc = dpool.tile([NPC_PAD, HID], BF16, tag="xrloc")

            def leaky(dst_ap, src_ap, slope, tag):
                # scalar-engine Lrelu ignores alpha on HW; use max(x, a*x)
                tmp = epool.tile(list(src_ap.shape), src_ap.dtype, tag=tag)
                nc.scalar.mul(tmp[:], src_ap, slope)
                nc.vector.tensor_tensor(out=dst_ap, in0=src_ap,
                                        in1=tmp[:], op=ALU.max)

            def layer(li, lhsT_tile, w_t, att_t, b_t, h_out, apply_leaky):
                nc.vector.memset(acc_t[:], 0.0)
                xl_sb = npool.tile([128, NTN, HID], BF16, tag="xlsb")
                xr_sb = npool.tile([128, NTN, HID], BF16, tag="xrsb")
                for t in range(NTN):
                    ps = mmps.tile([128, 2 * HID], F32, tag="nodeps")
                    nc.tensor.matmul(ps[:], lhsT_tile[:, t * 128:(t + 1) * 128],
                                     w_t[:], start=True, stop=True)
                    nc.scalar.copy(xl_sb[:, t, :], ps[:, :HID])
                    nc.scalar.copy(xr_sb[:, t, :], ps[:, HID:])
                # p-major DRAM tables (contiguous copies)
                nc.gpsimd.dma_start(cin[:], xl_sb[:].rearrange("p t f -> p (t f)"))
                nc.gpsimd.dma_start(
                    xr_loc[:].rearrange("(p t) f -> p (t f)", p=128),
                    xr_sb[:].rearrange("p t f -> p (t f)"))
                nc.gpsimd.collective_compute(
                    "AllGather", mybir.AluOpType.bypass,
                    replica_groups=[list(range(NCORES))],
                    ins=[cin.opt()], outs=[xl_full.opt()])

                cur = None  # (win, part, psum_tile)
                for ci in range(n_chunks):
                    t0 = ci * CHUNK_TILES
                    t1 = min(t0 + CHUNK_TILES, t_tot)
                    nt = t1 - t0

                    xj = epool.tile([128, CHUNK_TILES, HID], BF16, tag="xj")
                    xrd = epool.tile([128, CHUNK_TILES, HID], BF16, tag="xrd")
                    s_t = epool.tile([128, CHUNK_TILES, WIN], BF16, tag="smat")
                    nc.sync.dma_start(s_t[:, :nt, :], smat_d[:, t0:t1, :])
                    idxj_t = epool.tile([128, CHUNK_TILES * 8], I16, tag="idxjc")
                    idxd_t = epool.tile([128, CHUNK_TILES * 8], I16, tag="idxdc")
                    nc.sync.dma_start(idxj_t[:, :nt * 8], idxj_d[:, t0 * 8:t1 * 8])
                    nc.sync.dma_start(idxd_t[:, :nt * 8], idxd_d[:, t0 * 8:t1 * 8])

                    spans = []
                    if t0 < t_a:
                        spans.append((t0, min(t1, t_a), 0))
                    if t1 > t_a:
                        spans.append((max(t0, t_a), t1, 1))
                    for (sa, sb_, part) in spans:
                        n_i = (sb_ - sa) * TILE
                        tab = xl_full[SPLIT:NP_GLOB, :] if part else \
                            xl_full[0:SPLIT, :]
                        nc.gpsimd.dma_gather(
                            out_ap=xj[:, sa - t0:sb_ - t0, :], in_ap=tab,
                            idxs_ap=idxj_t[:, (sa - t0) * 8:(sb_ - t0) * 8],
                            num_idxs=n_i, num_idxs_reg=n_i, elem_size=HID,
                            single_packet=False)
                    nc.gpsimd.dma_gather(
                        out_ap=xrd[:, :nt, :], in_ap=xr_loc[:],
                        idxs_ap=idxd_t[:, :nt * 8],
                        num_idxs=nt * TILE, num_idxs_reg=nt * TILE,
                        elem_size=HID, single_packet=False)

                    ev = epool.tile([128, CHUNK_TILES, HID], BF16, tag="ev")
                    nc.vector.tensor_tensor(out=ev[:, :nt, :], in0=xj[:, :nt, :],
                                            in1=xrd[:, :nt, :], op=ALU.add)
                    leaky(ev[:, :nt, :], ev[:, :nt, :], ATT_SLOPE, "lk")

                    alph = epool.tile([128, CHUNK_TILES, 2], F32, tag="alph")
                    prod = epool.tile([128, CHUNK_TILES, HID], BF16, tag="lk")
                    nc.vector.tensor_tensor(
                        out=prod[:, :nt, :], in0=ev[:, :nt, :],
                        in1=att_t[:].unsqueeze(1).to_broadcast([128, nt, HID]),
                        op=ALU.mult)
                    nc.vector.tensor_reduce(
                        out=alph[:, :nt, :],
                        in_=prod[:, :nt, :].rearrange("p t (h c) -> p t h c", h=2),
                        axis=mybir.AxisListType.X, op=ALU.add)
                    wgt = epool.tile([128, CHUNK_TILES, 2], F32, tag="wgt")
                    nc.scalar.activation(wgt[:, :nt, :], alph[:, :nt, :], AF.Exp)
                    wgt_bf = epool.tile([128, CHUNK_TILES, 2], BF16, tag="wgtbf")
                    nc.vector.tensor_copy(wgt_bf[:, :nt, :], wgt[:, :nt, :])

                    rhs = epool.tile([128, CHUNK_TILES, HID + 2], BF16, tag="rhs")
                    nc.vector.tensor_tensor(
                        out=rhs[:, :nt, :HID].rearrange("p t (h c) -> p t h c", h=2),
                        in0=xj[:, :nt, :].rearrange("p t (h c) -> p t h c", h=2),
                        in1=wgt_bf[:, :nt, :].unsqueeze(3).to_broadcast(
                            [128, nt, 2, C]),
                        op=ALU.mult)
                    nc.vector.tensor_copy(rhs[:, :nt, HID:], wgt_bf[:, :nt, :])

                    for t in range(t0, t1):
                        wi, part = tile_win[t], tile_part[t]
                        if cur is None or (cur[0], cur[1]) != (wi, part):
                            winps = wps.tile([WIN, HID + 2], F32, tag="winps",
                                             name=f"winps_{li}_{wi}_{part}")
                            cur = (wi, part, winps)
                        first = (t == 0) or (tile_win[t - 1], tile_part[t - 1]) != (wi, part)
                        last = (t == t_tot - 1) or \
                            (tile_win[t + 1], tile_part[t + 1]) != (wi, part)
                        nc.tensor.matmul(cur[2][:], s_t[:, t - t0, :],
                                         rhs[:, t - t0, :],
                                         start=first, stop=last)
                        if last:
                            dst = acc_t[(wi % 2) * WIN:(wi % 2) * WIN + WIN,
                                        wi // 2, :]
                            nc.vector.tensor_tensor(out=dst, in0=dst,
                                                    in1=cur[2][:], op=ALU.add)
                            cur = None

                den = npool.tile([128, WINS // 2, 2], F32, tag="den")
                nc.vector.tensor_scalar_add(den[:], acc_t[:, :, HID:], 1e-16)
                rden = npool.tile([128, WINS // 2, 2], F32, tag="rden")
                nc.vector.reciprocal(rden[:], den[:])
                nc.vector.tensor_tensor(
                    out=h_out[:].rearrange("p t (h c) -> p t h c", h=2),
                    in0=acc_t[:, :, :HID].rearrange("p t (h c) -> p t h c", h=2),
                    in1=rden[:].unsqueeze(3).to_broadcast([128, WINS // 2, 2, C]),
                    op=ALU.mult)
                nc.vector.tensor_tensor(
                    out=h_out[:], in0=h_out[:],
                    in1=b_t[:].unsqueeze(1).to_broadcast([128, WINS // 2, HID]),
                    op=ALU.add)
                if apply_leaky:
                    leaky(h_out[:], h_out[:], OUT_SLOPE, "lkf")

            # ---------------- layer 1 ----------------
            layer(0, xT_t, w1_t, att1_t, b1_t, h1_t, apply_leaky=True)

            # h1^T tiles for layer 2 (PE transpose)
            for t in range(NTN):
                tp = mmps.tile([128, 128], BF16, tag="tps")
                nc.tensor.transpose(tp[:], h1_t[:, t, :], id_t[:])
                nc.scalar.copy(h1T_t[:, t * 128:(t + 1) * 128], tp[:])

            # ---------------- layer 2 ----------------
            h2_t = npool.tile([128, NTN, HID], F32, tag="h2")
            layer(1, h1T_t, w2_t, att2_t, b2_t, h2_t, apply_leaky=False)

            nc.vector.tensor_tensor(out=h2_t[:], in0=h2_t[:], in1=h1_t[:],
                                    op=ALU.add)
            nc.sync.dma_start(out_d[:], h2_t[:])

    nc.compile()
    return nc


def make_inputs(x, edge_index, w_l1, w_r1, att1, b1, w_l2, w_r2, att2, b2):
    """Host-side prep: returns (meta, in_maps)."""
    meta, per_core = prep_edges(edge_index)
    x = np.asarray(x, dtype=np.float32)
    ident = np.eye(128, dtype=np.float32).astype(BF)

    def wcat(wl, wr):
        return np.concatenate([np.asarray(wl).T, np.asarray(wr).T],
                              axis=1).astype(BF)

    att_bc = lambda a: np.tile(np.asarray(a).reshape(1, HID), (128, 1)).astype(BF)
    b_bc = lambda b: np.tile(np.asarray(b).reshape(1, HID),
                             (128, 1)).astype(np.float32)

    w1 = wcat(w_l1, w_r1)
    w2 = wcat(w_l2, w_r2)
    a1, a2 = att_bc(att1), att_bc(att2)
    bb1, bb2 = b_bc(b1), b_bc(b2)

    in_maps = []
    for k in range(NCORES):
        xs = np.zeros((NPC_PAD, HID), dtype=np.float32)
        xs[:NPC] = x[k * NPC:(k + 1) * NPC]
        in_maps.append({
            "xT": np.ascontiguousarray(xs.T).astype(BF),
            "w1T": w1, "w2T": w2, "attbc1": a1, "attbc2": a2,
            "bias1": bb1, "bias2": bb2, "ident": ident,
            **per_core[k],
        })
    return meta, in_maps


def kernel(**inputs):
    from concourse.bass_utils import run_bass_kernel_spmd

    meta, in_maps = make_inputs(**inputs)
    nc = build_bass(meta)
    res = run_bass_kernel_spmd(nc, in_maps, list(range(NCORES)))
    outs = []
    for k in range(NCORES):
        o = res.results[k]["out"]          # [128, NTN, HID]
        outs.append(o.transpose(1, 0, 2).reshape(NPC_PAD, HID)[:NPC])
    return np.concatenate(outs, axis=0).astype(np.float32)



# revision 7
# speedup vs baseline: 1.7380x; 1.7380x over previous
"""GATv2 (2-layer) Trainium2 Bass kernel, 8-core SPMD. v2.

Strategy (node-partitioned): core k owns dst nodes [k*6250, (k+1)*6250).
Host sorts each core's edges by (src-table-half, dst-window), windows are
128 dst nodes (== one node tile). Per layer:
 - Node phase: PE transforms the core's node slice (lhsT = x^T tiles,
   rhs = [w_l^T | w_r^T]); x_l slice is written p-major to DRAM and
   AllGathered into the global gather table; x_r stays in SBUF as
   [128, WINS, HID] (window-node on partitions).
 - Edge phase (the hot loop), per 16-tile chunk:
   * x_j = dma_gather(xl_table, idx) -- the ONLY per-edge DMA gather
     (v1 also gathered x_r per edge; Q7 SWDGE descriptor generation at
     ~8ns/row was 72%% of runtime, so halving descriptors is the win).
   * x_rd per edge via PE: psum = S^T.T @ xr_win (S^T [128n,128e] bf16
     from host; zero cols for pad slots).
   * S [128e,128n] generated on-chip: is_equal(dstw, iota) (dstw = -1
     for pads -> zero rows, so pad edges never touch the segment sums).
   * alpha = sum_c leakyrelu(x_j + x_rd, 0.2)*att, w = exp(alpha)
     (softmax max-shift cancels; alphas are small so exp is safe).
   * Segment sum on PE: psum[128n, 130] += S.T @ [w*x_j | w] accumulated
     over each (window, part) group; groups dumped into an SBUF acc.
 - Finalize: h = acc[:, :, :128]/(denom+1e-16) + bias (+leaky 0.01 for
   layer 1). Layer 2 reuses the identical edge structure and the same
   DRAM table (same row mapping). Output = h1 + h2 of the owned slice.
"""
import numpy as np
import ml_dtypes

BF = ml_dtypes.bfloat16

N = 50000
E = 640000
HID = 128
HEADS = 2
C = 64
ATT_SLOPE = 0.2
OUT_SLOPE = 0.01
NCORES = 8
TILE = 128
WIN = 128
SPLIT = 32768
CHUNK_TILES = 16


def configure(n=50000, e=640000):
    global N, E, NPC, NPC_PAD, WINS, NTN, NP_GLOB
    N, E = n, e
    NPC = N // NCORES
    NPC_PAD = ((NPC + TILE - 1) // TILE) * TILE
    NTN = NPC_PAD // TILE
    WINS = NPC_PAD // WIN
    NP_GLOB = NPC_PAD * NCORES


configure()


def _pack_idx16(idx):
    n = len(idx)
    cols = -(-n // 16)
    buf = np.zeros((cols, 16), dtype=np.int16)
    buf.reshape(-1)[:n] = idx.astype(np.int16)
    return np.tile(buf.T, (8, 1))


def prep_edges(edge_index):
    src = np.asarray(edge_index[0], dtype=np.int64)
    dst = np.asarray(edge_index[1], dtype=np.int64)
    core_of = dst // NPC
    gp_all = (src // NPC) * NPC_PAD + (src % NPC)
    # p-major row in the gather table: node (k, loc) -> k*NPC_PAD + (loc%128)*NTN + loc//128
    row_all = (gp_all // NPC_PAD) * NPC_PAD + (gp_all % NPC_PAD % 128) * NTN \
        + (gp_all % NPC_PAD) // 128

    cores = []
    cnt = np.zeros((NCORES, WINS, 2), dtype=np.int64)
    for k in range(NCORES):
        m = core_of == k
        rj = row_all[m]
        dl = dst[m] - k * NPC
        order = np.argsort(dl, kind="stable")
        rj, dl = rj[order], dl[order]
        w = dl // WIN
        a = rj < SPLIT
        cnt[k, :, 0] = np.bincount(w[a], minlength=WINS)
        cnt[k, :, 1] = np.bincount(w[~a], minlength=WINS)
        cores.append((rj, dl, w, a))

    budget = (-(-cnt // TILE)).max(axis=0)      # [WINS, 2]
    tile_win, tile_part = [], []
    for part in (0, 1):
        for wi in range(WINS):
            tile_win += [wi] * int(budget[wi, part])
            tile_part += [part] * int(budget[wi, part])
    t_tot = len(tile_win)
    t_a = int(budget[:, 0].sum())

    per_core = []
    for k in range(NCORES):
        rj, dl, w, a = cores[k]
        idx_j = np.zeros(t_tot * TILE, dtype=np.int16)
        dstw = np.full(t_tot * TILE, -1.0, dtype=np.float32)
        pos = 0
        for part in (0, 1):
            mp = a if part == 0 else ~a
            for wi in range(WINS):
                sel = mp & (w == wi)
                r_sel, d_sel = rj[sel], dl[sel]
                n_real = len(r_sel)
                n_slot = int(budget[wi, part]) * TILE
                idx_j[pos:pos + n_real] = (r_sel - (SPLIT if part else 0)).astype(np.int16)
                dstw[pos:pos + n_real] = (d_sel - wi * WIN).astype(np.float32)
                pos += n_slot
        assert pos == t_tot * TILE

        dstw_slots = dstw.reshape(t_tot, TILE)          # [t, e]
        sT = (np.arange(WIN)[:, None, None] ==
              dstw_slots[None, :, :]).astype(BF)        # [n, t, e]
        per_core.append({
            "idxJ": _pack_idx16(idx_j),
            "dstw": np.ascontiguousarray(dstw_slots.T).astype(BF),   # [128, t]
            "smatT": np.ascontiguousarray(sT),
        })

    meta = {"t_tot": t_tot, "t_a": t_a,
            "tile_win": tile_win, "tile_part": tile_part}
    return meta, per_core


def build_bass(meta):
    from concourse import bacc, mybir, tile

    F32, BF16, I16 = mybir.dt.float32, mybir.dt.bfloat16, mybir.dt.int16
    AF = mybir.ActivationFunctionType
    ALU = mybir.AluOpType

    t_tot, t_a = meta["t_tot"], meta["t_a"]
    tile_win, tile_part = meta["tile_win"], meta["tile_part"]
    n_chunks = -(-t_tot // CHUNK_TILES)

    nc = bacc.Bacc("TRN2", target_bir_lowering=False, debug=False,
                   num_devices=NCORES)

    xT_d = nc.dram_tensor("xT", [HID, NPC_PAD], BF16, kind="ExternalInput")
    w1_d = nc.dram_tensor("w1T", [HID, 2 * HID], BF16, kind="ExternalInput")
    w2_d = nc.dram_tensor("w2T", [HID, 2 * HID], BF16, kind="ExternalInput")
    att1_d = nc.dram_tensor("attbc1", [128, HID], BF16, kind="ExternalInput")
    att2_d = nc.dram_tensor("attbc2", [128, HID], BF16, kind="ExternalInput")
    b1_d = nc.dram_tensor("bias1", [128, HID], F32, kind="ExternalInput")
    b2_d = nc.dram_tensor("bias2", [128, HID], F32, kind="ExternalInput")
    id_d = nc.dram_tensor("ident", [128, 128], BF16, kind="ExternalInput")
    iota_d = nc.dram_tensor("iota", [128, WIN], BF16, kind="ExternalInput")
    idxj_d = nc.dram_tensor("idxJ", [128, t_tot * 8], I16, kind="ExternalInput")
    dstw_d = nc.dram_tensor("dstw", [128, t_tot], BF16, kind="ExternalInput")
    smatT_d = nc.dram_tensor("smatT", [WIN, t_tot, TILE], BF16, kind="ExternalInput")
    out_d = nc.dram_tensor("out", [128, WINS, HID], F32, kind="ExternalOutput")

    with tile.TileContext(nc) as tc:
        with (
            tc.tile_pool(name="const", bufs=1) as cpool,
            tc.tile_pool(name="node", bufs=1) as npool,
            tc.tile_pool(name="nstage", bufs=1) as spool,
            tc.tile_pool(name="edge", bufs=2) as epool,
            tc.tile_pool(name="mmps", bufs=2, space="PSUM") as mmps,
            tc.tile_pool(name="wps", bufs=2, space="PSUM") as wps,
            tc.tile_pool(name="dram", bufs=1, space="DRAM") as dpool,
        ):
            w1_t = cpool.tile([HID, 2 * HID], BF16, tag="w1")
            w2_t = cpool.tile([HID, 2 * HID], BF16, tag="w2")
            att1_t = cpool.tile([128, HID], BF16, tag="att1")
            att2_t = cpool.tile([128, HID], BF16, tag="att2")
            b1_t = cpool.tile([128, HID], F32, tag="b1")
            b2_t = cpool.tile([128, HID], F32, tag="b2")
            id_t = cpool.tile([128, 128], BF16, tag="ident")
            iota_t = cpool.tile([128, WIN], BF16, tag="iota")
            idxj_t = cpool.tile([128, t_tot * 8], I16, tag="idxj")
            dstw_t = cpool.tile([128, t_tot], BF16, tag="dstw")

            for tdst, tsrc in ((w1_t, w1_d), (w2_t, w2_d), (att1_t, att1_d),
                               (att2_t, att2_d), (b1_t, b1_d), (b2_t, b2_d),
                               (id_t, id_d), (iota_t, iota_d),
                               (idxj_t, idxj_d), (dstw_t, dstw_d)):
                nc.sync.dma_start(tdst[:], tsrc[:])

            xT_t = npool.tile([HID, NPC_PAD], BF16, tag="xT")
            nc.sync.dma_start(xT_t[:], xT_d[:])

            h1_t = npool.tile([128, WINS, HID], BF16, tag="h1")
            h1T_t = xT_t  # xT is dead after layer-1 node phase; reuse the slot
            acc_t = npool.tile([128, WINS, HID + 2], F32, tag="acc")
            xr_t = npool.tile([128, WINS, HID], BF16, tag="xr")

            cin = dpool.tile([128, NPC_PAD], BF16, tag="cin")
            xl_full = dpool.tile([NP_GLOB, HID], BF16, tag="xlfull")

            def leaky(dst_ap, src_ap, slope, tag, shape, dt_, pool=None):
                tmp = (pool or epool).tile(shape, dt_, tag=tag)
                nc.scalar.mul(tmp[:], src_ap, slope)
                nc.vector.tensor_tensor(out=dst_ap, in0=src_ap,
                                        in1=tmp[:], op=ALU.max)

            def layer(li, lhsT_tile, w_t, att_t, b_t, h_out, apply_leaky):
                # ---- node phase: xl slice -> DRAM table, xr -> SBUF ----
                xl_sb = spool.tile([128, NTN, HID], BF16, tag="xlsb")
                for t in range(NTN):
                    ps = mmps.tile([128, 2 * HID], F32, tag="nodeps")
                    nc.tensor.matmul(ps[:], lhsT_tile[:, t * 128:(t + 1) * 128],
                                     w_t[:], start=True, stop=True)
                    nc.scalar.copy(xl_sb[:, t, :], ps[:, :HID])
                    nc.vector.tensor_copy(xr_t[:, t, :], ps[:, HID:])
                nc.sync.dma_start(cin[:], xl_sb[:].rearrange("p t f -> p (t f)"))
                nc.gpsimd.collective_compute(
                    "AllGather", mybir.AluOpType.bypass,
                    replica_groups=[list(range(NCORES))],
                    ins=[cin.opt()], outs=[xl_full.opt()])

                nc.vector.memset(acc_t[:], 0.0)

                # ---- edge phase ----
                cur_ps = None
                for ci in range(n_chunks):
                    t0 = ci * CHUNK_TILES
                    t1 = min(t0 + CHUNK_TILES, t_tot)
                    nt = t1 - t0

                    xj = epool.tile([128, CHUNK_TILES, HID], BF16, tag="xj")
                    spans = []
                    if t0 < t_a:
                        spans.append((t0, min(t1, t_a), 0))
                    if t1 > t_a:
                        spans.append((max(t0, t_a), t1, 1))
                    for (sa, sb_, part) in spans:
                        n_i = (sb_ - sa) * TILE
                        tab = xl_full[SPLIT:NP_GLOB, :] if part else \
                            xl_full[0:SPLIT, :]
                        nc.gpsimd.dma_gather(
                            out_ap=xj[:, sa - t0:sb_ - t0, :], in_ap=tab,
                            idxs_ap=idxj_t[:, sa * 8:sb_ * 8],
                            num_idxs=n_i, num_idxs_reg=n_i, elem_size=HID,
                            single_packet=False)

                    sT_t = epool.tile([WIN, CHUNK_TILES, TILE], BF16, tag="smt")
                    nc.sync.dma_start(sT_t[:, :nt, :], smatT_d[:, t0:t1, :])

                    # S on-chip: [128e, nt, 128n] = (dstw == iota)
                    s_t = epool.tile([128, CHUNK_TILES, WIN], BF16, tag="smat")
                    nc.vector.tensor_tensor(
                        out=s_t[:, :nt, :],
                        in0=dstw_t[:, t0:t1].unsqueeze(2).to_broadcast(
                            [128, nt, WIN]),
                        in1=iota_t[:].unsqueeze(1).to_broadcast([128, nt, WIN]),
                        op=ALU.is_equal)

                    # x_rd per edge via PE, evacuate psum -> sbuf (ACT)
                    xrd = epool.tile([128, CHUNK_TILES, HID], BF16, tag="xrd")
                    for t in range(t0, t1):
                        wi = tile_win[t]
                        xps = mmps.tile([128, HID], F32, tag="xrdps")
                        nc.tensor.matmul(xps[:], sT_t[:, t - t0, :],
                                         xr_t[:, wi, :], start=True, stop=True)
                        nc.scalar.copy(xrd[:, t - t0, :], xps[:])

                    ev = epool.tile([128, CHUNK_TILES, HID], BF16, tag="ev")
                    nc.vector.tensor_tensor(out=ev[:, :nt, :], in0=xj[:, :nt, :],
                                            in1=xrd[:, :nt, :], op=ALU.add)
                    leaky(ev[:, :nt, :], ev[:, :nt, :], ATT_SLOPE, "lk",
                          [128, CHUNK_TILES, HID], BF16)

                    alph = epool.tile([128, CHUNK_TILES, 2], F32, tag="alph")
                    prod = epool.tile([128, CHUNK_TILES, HID], BF16, tag="lk")
                    nc.vector.tensor_tensor(
                        out=prod[:, :nt, :], in0=ev[:, :nt, :],
                        in1=att_t[:].unsqueeze(1).to_broadcast([128, nt, HID]),
                        op=ALU.mult)
                    nc.vector.tensor_reduce(
                        out=alph[:, :nt, :],
                        in_=prod[:, :nt, :].rearrange("p t (h c) -> p t h c", h=2),
                        axis=mybir.AxisListType.X, op=ALU.add)
                    wgt = epool.tile([128, CHUNK_TILES, 2], F32, tag="wgt")
                    nc.scalar.activation(wgt[:, :nt, :], alph[:, :nt, :], AF.Exp)
                    wgt_bf = epool.tile([128, CHUNK_TILES, 2], BF16, tag="wgtbf")
                    nc.vector.tensor_copy(wgt_bf[:, :nt, :], wgt[:, :nt, :])

                    rhs = epool.tile([128, CHUNK_TILES, HID + 2], BF16, tag="rhs")
                    nc.vector.tensor_tensor(
                        out=rhs[:, :nt, :HID].rearrange("p t (h c) -> p t h c", h=2),
                        in0=xj[:, :nt, :].rearrange("p t (h c) -> p t h c", h=2),
                        in1=wgt_bf[:, :nt, :].unsqueeze(3).to_broadcast(
                            [128, nt, 2, C]),
                        op=ALU.mult)
                    nc.vector.tensor_copy(rhs[:, :nt, HID:], wgt_bf[:, :nt, :])

                    for t in range(t0, t1):
                        wi, part = tile_win[t], tile_part[t]
                        if cur_ps is None:
                            cur_ps = wps.tile([WIN, HID + 2], F32, tag="winps",
                                              name=f"winps_{li}_{wi}_{part}")
                        first = (t == 0) or (tile_win[t - 1], tile_part[t - 1]) != (wi, part)
                        last = (t == t_tot - 1) or \
                            (tile_win[t + 1], tile_part[t + 1]) != (wi, part)
                        nc.tensor.matmul(cur_ps[:], s_t[:, t - t0, :],
                                         rhs[:, t - t0, :],
                                         start=first, stop=last)
                        if last:
                            dst = acc_t[:, wi, :HID + 2]
                            nc.vector.tensor_tensor(out=dst, in0=dst,
                                                    in1=cur_ps[:], op=ALU.add)
                            cur_ps = None

                # ---- finalize ----
                den = npool.tile([128, WINS, 2], F32, tag="den")
                nc.vector.tensor_scalar_add(den[:], acc_t[:, :, HID:], 1e-16)
                rden = npool.tile([128, WINS, 2], F32, tag="rden")
                nc.vector.reciprocal(rden[:], den[:])
                nc.vector.tensor_tensor(
                    out=h_out[:].rearrange("p t (h c) -> p t h c", h=2),
                    in0=acc_t[:, :, :HID].rearrange("p t (h c) -> p t h c", h=2),
                    in1=rden[:].unsqueeze(3).to_broadcast([128, WINS, 2, C]),
                    op=ALU.mult)
                nc.vector.tensor_tensor(
                    out=h_out[:], in0=h_out[:],
                    in1=b_t[:].unsqueeze(1).to_broadcast([128, WINS, HID]),
                    op=ALU.add)
                if apply_leaky:
                    leaky(h_out[:], h_out[:], OUT_SLOPE, "lkf",
                          [128, WINS, HID], h_out.dtype, pool=npool)

            # ---------------- layer 1 ----------------
            layer(0, xT_t, w1_t, att1_t, b1_t, h1_t, apply_leaky=True)

            # h1^T tiles for layer 2 (PE transpose)
            for t in range(NTN):
                tp = mmps.tile([128, 128], BF16, tag="tps")
                nc.tensor.transpose(tp[:], h1_t[:, t, :], id_t[:])
                nc.scalar.copy(h1T_t[:, t * 128:(t + 1) * 128], tp[:])

            # ---------------- layer 2 ----------------
            h2_t = npool.tile([128, WINS, HID], F32, tag="h2")
            layer(1, h1T_t, w2_t, att2_t, b2_t, h2_t, apply_leaky=False)

            nc.vector.tensor_tensor(out=h2_t[:], in0=h2_t[:], in1=h1_t[:],
                                    op=ALU.add)
            nc.sync.dma_start(out_d[:], h2_t[:])

    nc.compile()
    return nc


def make_inputs(x, edge_index, w_l1, w_r1, att1, b1, w_l2, w_r2, att2, b2):
    """Host-side prep: returns (meta, in_maps)."""
    meta, per_core = prep_edges(edge_index)
    x = np.asarray(x, dtype=np.float32)
    ident = np.eye(128, dtype=np.float32).astype(BF)
    iota = np.tile(np.arange(WIN, dtype=np.float32), (128, 1)).astype(BF)

    def wcat(wl, wr):
        return np.concatenate([np.asarray(wl).T, np.asarray(wr).T],
                              axis=1).astype(BF)

    att_bc = lambda a: np.tile(np.asarray(a).reshape(1, HID), (128, 1)).astype(BF)
    b_bc = lambda b: np.tile(np.asarray(b).reshape(1, HID),
                             (128, 1)).astype(np.float32)

    w1 = wcat(w_l1, w_r1)
    w2 = wcat(w_l2, w_r2)
    a1, a2 = att_bc(att1), att_bc(att2)
    bb1, bb2 = b_bc(b1), b_bc(b2)

    in_maps = []
    for k in range(NCORES):
        xs = np.zeros((NPC_PAD, HID), dtype=np.float32)
        xs[:NPC] = x[k * NPC:(k + 1) * NPC]
        in_maps.append({
            "xT": np.ascontiguousarray(xs.T).astype(BF),
            "w1T": w1, "w2T": w2, "attbc1": a1, "attbc2": a2,
            "bias1": bb1, "bias2": bb2, "ident": ident, "iota": iota,
            **per_core[k],
        })
    return meta, in_maps


def kernel(**inputs):
    from concourse.bass_utils import run_bass_kernel_spmd

    meta, in_maps = make_inputs(**inputs)
    nc = build_bass(meta)
    res = run_bass_kernel_spmd(nc, in_maps, list(range(NCORES)))
    outs = []
    for k in range(NCORES):
        o = res.results[k]["out"]          # [128, WINS, HID]
        outs.append(o.transpose(1, 0, 2).reshape(NPC_PAD, HID)[:NPC])
    return np.concatenate(outs, axis=0).astype(np.float32)


# revision 11
# speedup vs baseline: 2.4640x; 1.4177x over previous
"""GATv2 (2-layer) Trainium2 Bass kernel, 8-core SPMD. v3.

Strategy (node-partitioned): core k owns dst nodes [k*6250, (k+1)*6250).
Host sorts each core's edges by (src-table-half, dst-window); windows are
128 dst nodes (== one node tile of the core's slice). Per layer:
 - Node phase: PE transforms the core's node slice (lhsT = x^T tiles,
   rhs = [w_l^T | w_r^T]); x_l slice is written p-major to DRAM and
   AllGathered into the global gather table; x_r stays in SBUF as
   [128, WINS, HID] (window-node on partitions).
 - Edge phase (the hot loop), per 32-tile chunk:
   * x_j = dma_gather(xl_table, idx) -- the ONLY per-edge DMA gather.
     Q7 SWDGE descriptor generation (~8ns/row, 2 Q7 cores, serialized)
     is the machine bottleneck, so v2/v3 eliminated the per-edge x_r
     gather and the rest of the pipeline hides under the gathers.
   * x_rd per edge via PE: psum = S^T.T @ xr_win (S^T [128n,128e] bf16
     from host; zero cols for pad slots). Evacuated 4 tiles per ACT
     copy (batched psum->sbuf).
   * alpha = sum_c leakyrelu(x_j + x_rd, 0.2)*att, w = exp(alpha)
     (softmax max-shift cancels; alphas are small so exp is safe).
   * Segment sum on PE: psum[128n, 130] += S.T @ [w*x_j | w] over each
     (window, part) tile group; groups dumped into an SBUF acc.
 - Finalize: h = acc[:, :, :128]/(denom+1e-16) + bias (+leaky 0.01 for
   layer 1). Layer 2 reuses the identical edge structure and the same
   DRAM table/row mapping. Output = h1 + h2 of the owned slice.
"""
import numpy as np
import ml_dtypes

BF = ml_dtypes.bfloat16

N = 50000
E = 640000
HID = 128
HEADS = 2
C = 64
ATT_SLOPE = 0.2
OUT_SLOPE = 0.01
NCORES = 8
TILE = 128
WIN = 128
SPLIT = 32768
CHUNK_TILES = 24
ATT_TILES = 16
XRD_BATCH = 4


def configure(n=50000, e=640000):
    global N, E, NPC, NPC_PAD, WINS, NTN, NP_GLOB
    N, E = n, e
    NPC = N // NCORES
    NPC_PAD = ((NPC + TILE - 1) // TILE) * TILE
    NTN = NPC_PAD // TILE
    WINS = NPC_PAD // WIN
    NP_GLOB = NPC_PAD * NCORES


configure()


def _pack_idx16(idx):
    n = len(idx)
    cols = -(-n // 16)
    buf = np.zeros((cols, 16), dtype=np.int16)
    buf.reshape(-1)[:n] = idx.astype(np.int16)
    return np.tile(buf.T, (8, 1))


def prep_edges(edge_index):
    src = np.asarray(edge_index[0], dtype=np.int64)
    dst = np.asarray(edge_index[1], dtype=np.int64)
    core_of = dst // NPC
    gp_all = (src // NPC) * NPC_PAD + (src % NPC)
    # p-major row in the gather table: node (k, loc) -> k*NPC_PAD + (loc%128)*NTN + loc//128
    row_all = (gp_all // NPC_PAD) * NPC_PAD + (gp_all % NPC_PAD % 128) * NTN \
        + (gp_all % NPC_PAD) // 128

    cores = []
    cnt = np.zeros((NCORES, WINS, 2), dtype=np.int64)
    for k in range(NCORES):
        m = core_of == k
        rj = row_all[m]
        dl = dst[m] - k * NPC
        order = np.argsort(dl, kind="stable")
        rj, dl = rj[order], dl[order]
        w = dl // WIN
        a = rj < SPLIT
        cnt[k, :, 0] = np.bincount(w[a], minlength=WINS)
        cnt[k, :, 1] = np.bincount(w[~a], minlength=WINS)
        cores.append((rj, dl, w, a))

    budget = (-(-cnt // TILE)).max(axis=0)      # [WINS, 2]
    tile_win, tile_part = [], []
    for part in (0, 1):
        for wi in range(WINS):
            tile_win += [wi] * int(budget[wi, part])
            tile_part += [part] * int(budget[wi, part])
    t_tot = len(tile_win)
    t_a = int(budget[:, 0].sum())

    per_core = []
    for k in range(NCORES):
        rj, dl, w, a = cores[k]
        idx_j = np.zeros(t_tot * TILE, dtype=np.int16)
        dstw = np.full(t_tot * TILE, -1.0, dtype=np.float32)
        pos = 0
        for part in (0, 1):
            mp = a if part == 0 else ~a
            for wi in range(WINS):
                sel = mp & (w == wi)
                r_sel, d_sel = rj[sel], dl[sel]
                n_real = len(r_sel)
                n_slot = int(budget[wi, part]) * TILE
                idx_j[pos:pos + n_real] = (r_sel - (SPLIT if part else 0)).astype(np.int16)
                dstw[pos:pos + n_real] = (d_sel - wi * WIN).astype(np.float32)
                pos += n_slot
        assert pos == t_tot * TILE

        dstw_slots = dstw.reshape(t_tot, TILE)          # [t, e]
        sT = (np.arange(WIN)[:, None, None] ==
              dstw_slots[None, :, :]).astype(BF)        # [n, t, e]
        s_mat = sT.transpose(2, 1, 0)                   # [e, t, n]
        per_core.append({
            "idxJ": _pack_idx16(idx_j),
            "smat": np.ascontiguousarray(s_mat),
            "smatT": np.ascontiguousarray(sT),
        })

    meta = {"t_tot": t_tot, "t_a": t_a,
            "tile_win": tile_win, "tile_part": tile_part}
    return meta, per_core


def build_bass(meta):
    from concourse import bacc, mybir, tile

    F32, BF16, I16 = mybir.dt.float32, mybir.dt.bfloat16, mybir.dt.int16
    AF = mybir.ActivationFunctionType
    ALU = mybir.AluOpType

    t_tot, t_a = meta["t_tot"], meta["t_a"]
    tile_win, tile_part = meta["tile_win"], meta["tile_part"]
    n_chunks = -(-t_tot // CHUNK_TILES)

    nc = bacc.Bacc("TRN2", target_bir_lowering=False, debug=False,
                   num_devices=NCORES)

    xT_d = nc.dram_tensor("xT", [HID, NPC_PAD], BF16, kind="ExternalInput")
    w1_d = nc.dram_tensor("w1T", [HID, 2 * HID], BF16, kind="ExternalInput")
    w2_d = nc.dram_tensor("w2T", [HID, 2 * HID], BF16, kind="ExternalInput")
    att1_d = nc.dram_tensor("attbc1", [128, ATT_TILES * HID], BF16,
                            kind="ExternalInput")
    att2_d = nc.dram_tensor("attbc2", [128, ATT_TILES * HID], BF16,
                            kind="ExternalInput")
    b1_d = nc.dram_tensor("bias1", [128, HID], F32, kind="ExternalInput")
    b2_d = nc.dram_tensor("bias2", [128, HID], F32, kind="ExternalInput")
    id_d = nc.dram_tensor("ident", [128, 128], BF16, kind="ExternalInput")
    idxj_d = nc.dram_tensor("idxJ", [128, t_tot * 8], I16, kind="ExternalInput")
    smat_d = nc.dram_tensor("smat", [128, t_tot, WIN], BF16, kind="ExternalInput")
    smatT_d = nc.dram_tensor("smatT", [WIN, t_tot, TILE], BF16, kind="ExternalInput")
    out_d = nc.dram_tensor("out", [128, WINS, HID], BF16, kind="ExternalOutput")

    with tile.TileContext(nc) as tc:
        with (
            tc.tile_pool(name="const", bufs=1) as cpool,
            tc.tile_pool(name="node", bufs=1) as npool,
            tc.tile_pool(name="nstage", bufs=1) as spool,
            tc.tile_pool(name="gat", bufs=3) as gpool,
            tc.tile_pool(name="edge", bufs=2) as epool,
            tc.tile_pool(name="mmps", bufs=2, space="PSUM") as mmps,
            tc.tile_pool(name="wps", bufs=2, space="PSUM") as wps,
            tc.tile_pool(name="dram", bufs=1, space="DRAM") as dpool,
        ):
            w1_t = cpool.tile([HID, 2 * HID], BF16, tag="w1")
            w2_t = cpool.tile([HID, 2 * HID], BF16, tag="w2")
            att1_t = cpool.tile([128, ATT_TILES, HID], BF16, tag="att1")
            att2_t = cpool.tile([128, ATT_TILES, HID], BF16, tag="att2")
            b1_t = cpool.tile([128, HID], F32, tag="b1")
            b2_t = cpool.tile([128, HID], F32, tag="b2")
            id_t = cpool.tile([128, 128], BF16, tag="ident")
            idxj_t = cpool.tile([128, t_tot * 8], I16, tag="idxj")

            for tdst, tsrc in ((w1_t, w1_d), (w2_t, w2_d),
                               (att1_t.rearrange("p t f -> p (t f)"), att1_d),
                               (att2_t.rearrange("p t f -> p (t f)"), att2_d),
                               (b1_t, b1_d), (b2_t, b2_d),
                               (id_t, id_d), (idxj_t, idxj_d)):
                nc.sync.dma_start(tdst[:], tsrc[:])

            xT_t = npool.tile([HID, NPC_PAD], BF16, tag="xT")
            nc.sync.dma_start(xT_t[:], xT_d[:])

            h1_t = npool.tile([128, WINS, HID], BF16, tag="h1")
            h1T_t = xT_t  # xT is dead after layer-1 node phase; reuse the slot
            acc_t = npool.tile([128, WINS, HID + 2], F32, tag="acc")
            xr_t = npool.tile([128, WINS, HID], BF16, tag="xr")

            cin = dpool.tile([128, NPC_PAD], BF16, tag="cin")
            xl_full = dpool.tile([NP_GLOB, HID], BF16, tag="xlfull")

            def leaky(dst_ap, src_ap, slope, tag, shape, dt_, pool, nt=None):
                tmp = pool.tile(shape, dt_, tag=tag)
                tv = tmp[:, :nt, :] if nt is not None else tmp[:]
                nc.scalar.mul(tv, src_ap, slope)
                nc.vector.tensor_tensor(out=dst_ap, in0=src_ap,
                                        in1=tv, op=ALU.max)

            def layer(li, lhsT_tile, w_t, att_t, b_t, h_out, apply_leaky):
                # ---- node phase: xl slice -> DRAM table, xr -> SBUF ----
                xl_sb = spool.tile([128, NTN, HID], BF16, tag="xlsb")
                for t in range(NTN):
                    ps = mmps.tile([128, 2 * HID], F32, tag="nodeps")
                    nc.tensor.matmul(ps[:], lhsT_tile[:, t * 128:(t + 1) * 128],
                                     w_t[:], start=True, stop=True)
                    nc.scalar.copy(xl_sb[:, t, :], ps[:, :HID])
                    nc.vector.tensor_copy(xr_t[:, t, :], ps[:, HID:])
                nc.sync.dma_start(cin[:], xl_sb[:].rearrange("p t f -> p (t f)"))
                nc.gpsimd.collective_compute(
                    "AllGather", mybir.AluOpType.bypass,
                    replica_groups=[list(range(NCORES))],
                    ins=[cin.opt()], outs=[xl_full.opt()])

                nc.vector.memset(acc_t[:], 0.0)

                # ---- edge phase ----
                cur_ps = None
                for ci in range(n_chunks):
                    t0 = ci * CHUNK_TILES
                    t1 = min(t0 + CHUNK_TILES, t_tot)
                    nt = t1 - t0

                    xj = gpool.tile([128, CHUNK_TILES, HID], BF16, tag="xj")
                    spans = []
                    if t0 < t_a:
                        spans.append((t0, min(t1, t_a), 0))
                    if t1 > t_a:
                        spans.append((max(t0, t_a), t1, 1))
                    for (sa, sb_, part) in spans:
                        n_i = (sb_ - sa) * TILE
                        tab = xl_full[SPLIT:NP_GLOB, :] if part else \
                            xl_full[0:SPLIT, :]
                        nc.gpsimd.dma_gather(
                            out_ap=xj[:, sa - t0:sb_ - t0, :], in_ap=tab,
                            idxs_ap=idxj_t[:, sa * 8:sb_ * 8],
                            num_idxs=n_i, num_idxs_reg=n_i, elem_size=HID,
                            single_packet=False)

                    sT_t = epool.tile([WIN, CHUNK_TILES, TILE], BF16, tag="smt")
                    nc.sync.dma_start(sT_t[:, :nt, :], smatT_d[:, t0:t1, :])
                    s_t = epool.tile([128, CHUNK_TILES, WIN], BF16, tag="smat")
                    nc.sync.dma_start(s_t[:, :nt, :], smat_d[:, t0:t1, :])

                    # x_rd per edge via PE; batched psum->sbuf evacuation
                    xrd = epool.tile([128, CHUNK_TILES, HID], BF16, tag="xrd")
                    for tb in range(t0, t1, XRD_BATCH):
                        te = min(tb + XRD_BATCH, t1)
                        xps = mmps.tile([128, XRD_BATCH, HID], F32, tag="xrdps")
                        for t in range(tb, te):
                            nc.tensor.matmul(xps[:, t - tb, :],
                                             sT_t[:, t - t0, :],
                                             xr_t[:, tile_win[t], :],
                                             start=True, stop=True)
                        nc.scalar.copy(xrd[:, tb - t0:te - t0, :],
                                       xps[:, :te - tb, :])

                    ev = epool.tile([128, CHUNK_TILES, HID], BF16, tag="ev")
                    nc.vector.tensor_tensor(out=ev[:, :nt, :], in0=xj[:, :nt, :],
                                            in1=xrd[:, :nt, :], op=ALU.add)
                    leaky(ev[:, :nt, :], ev[:, :nt, :], ATT_SLOPE, "lk",
                          [128, CHUNK_TILES, HID], BF16, epool, nt=nt)

                    alph = epool.tile([128, CHUNK_TILES, 2], F32, tag="alph")
                    prod = epool.tile([128, CHUNK_TILES, HID], BF16, tag="lk")
                    for j0 in range(0, nt, ATT_TILES):
                        je = min(j0 + ATT_TILES, nt)
                        nc.vector.tensor_tensor(
                            out=prod[:, j0:je, :], in0=ev[:, j0:je, :],
                            in1=att_t[:, :je - j0, :], op=ALU.mult)
                    nc.vector.tensor_reduce(
                        out=alph[:, :nt, :],
                        in_=prod[:, :nt, :].rearrange("p t (h c) -> p t h c", h=2),
                        axis=mybir.AxisListType.X, op=ALU.add)
                    wgt_bf = epool.tile([128, CHUNK_TILES, 2], BF16, tag="wgtbf")
                    nc.scalar.activation(wgt_bf[:, :nt, :], alph[:, :nt, :], AF.Exp)

                    rhs = epool.tile([128, CHUNK_TILES, HID + 2], BF16, tag="rhs")
                    nc.vector.tensor_tensor(
                        out=rhs[:, :nt, :HID].rearrange("p t (h c) -> p t h c", h=2),
                        in0=xj[:, :nt, :].rearrange("p t (h c) -> p t h c", h=2),
                        in1=wgt_bf[:, :nt, :].unsqueeze(3).to_broadcast(
                            [128, nt, 2, C]),
                        op=ALU.mult)
                    nc.vector.tensor_copy(rhs[:, :nt, HID:], wgt_bf[:, :nt, :])

                    for t in range(t0, t1):
                        wi, part = tile_win[t], tile_part[t]
                        if cur_ps is None:
                            cur_ps = wps.tile([WIN, HID + 2], F32, tag="winps",
                                              name=f"winps_{li}_{wi}_{part}")
                        first = (t == 0) or (tile_win[t - 1], tile_part[t - 1]) != (wi, part)
                        last = (t == t_tot - 1) or \
                            (tile_win[t + 1], tile_part[t + 1]) != (wi, part)
                        nc.tensor.matmul(cur_ps[:], s_t[:, t - t0, :],
                                         rhs[:, t - t0, :],
                                         start=first, stop=last)
                        if last:
                            dst = acc_t[:, wi, :HID + 2]
                            nc.vector.tensor_tensor(out=dst, in0=dst,
                                                    in1=cur_ps[:], op=ALU.add)
                            cur_ps = None

                # ---- finalize ----
                den = npool.tile([128, WINS, 2], F32, tag="den")
                nc.vector.tensor_scalar_add(den[:], acc_t[:, :, HID:], 1e-16)
                rden = npool.tile([128, WINS, 2], F32, tag="rden")
                nc.vector.reciprocal(rden[:], den[:])
                nc.vector.tensor_tensor(
                    out=h_out[:].rearrange("p t (h c) -> p t h c", h=2),
                    in0=acc_t[:, :, :HID].rearrange("p t (h c) -> p t h c", h=2),
                    in1=rden[:].unsqueeze(3).to_broadcast([128, WINS, 2, C]),
                    op=ALU.mult)
                nc.vector.tensor_tensor(
                    out=h_out[:], in0=h_out[:],
                    in1=b_t[:].unsqueeze(1).to_broadcast([128, WINS, HID]),
                    op=ALU.add)
                if apply_leaky:
                    leaky(h_out[:], h_out[:], OUT_SLOPE, "xlsb",
                          [128, NTN, HID], BF16, spool)

            # ---------------- layer 1 ----------------
            layer(0, xT_t, w1_t, att1_t, b1_t, h1_t, apply_leaky=True)

            # h1^T tiles for layer 2 (PE transpose)
            for t in range(NTN):
                tp = mmps.tile([128, 128], BF16, tag="tps")
                nc.tensor.transpose(tp[:], h1_t[:, t, :], id_t[:])
                nc.scalar.copy(h1T_t[:, t * 128:(t + 1) * 128], tp[:])

            # ---------------- layer 2 ----------------
            h2_t = npool.tile([128, WINS, HID], BF16, tag="h2")
            layer(1, h1T_t, w2_t, att2_t, b2_t, h2_t, apply_leaky=False)

            nc.vector.tensor_tensor(out=h2_t[:], in0=h2_t[:], in1=h1_t[:],
                                    op=ALU.add)
            nc.sync.dma_start(out_d[:], h2_t[:])

    nc.compile()
    return nc


def make_inputs(x, edge_index, w_l1, w_r1, att1, b1, w_l2, w_r2, att2, b2):
    """Host-side prep: returns (meta, in_maps)."""
    meta, per_core = prep_edges(edge_index)
    x = np.asarray(x, dtype=np.float32)
    ident = np.eye(128, dtype=np.float32).astype(BF)

    def wcat(wl, wr):
        return np.concatenate([np.asarray(wl).T, np.asarray(wr).T],
                              axis=1).astype(BF)

    def att_tiled(a):
        row = np.asarray(a).reshape(1, HID)
        return np.tile(row, (128, ATT_TILES)).astype(BF)

    b_bc = lambda b: np.tile(np.asarray(b).reshape(1, HID),
                             (128, 1)).astype(np.float32)

    w1 = wcat(w_l1, w_r1)
    w2 = wcat(w_l2, w_r2)
    a1, a2 = att_tiled(att1), att_tiled(att2)
    bb1, bb2 = b_bc(b1), b_bc(b2)

    in_maps = []
    for k in range(NCORES):
        xs = np.zeros((NPC_PAD, HID), dtype=np.float32)
        xs[:NPC] = x[k * NPC:(k + 1) * NPC]
        in_maps.append({
            "xT": np.ascontiguousarray(xs.T).astype(BF),
            "w1T": w1, "w2T": w2, "attbc1": a1, "attbc2": a2,
            "bias1": bb1, "bias2": bb2, "ident": ident,
            **per_core[k],
        })
    return meta, in_maps


def kernel(**inputs):
    from concourse.bass_utils import run_bass_kernel_spmd

    meta, in_maps = make_inputs(**inputs)
    nc = build_bass(meta)
    res = run_bass_kernel_spmd(nc, in_maps, list(range(NCORES)))
    outs = []
    for k in range(NCORES):
        o = res.results[k]["out"]          # [128, WINS, HID]
        outs.append(o.transpose(1, 0, 2).reshape(NPC_PAD, HID)[:NPC])
    return np.concatenate(outs, axis=0).astype(np.float32)


# revision 13
# speedup vs baseline: 2.5905x; 1.0513x over previous
"""GATv2 (2-layer) Trainium2 Bass kernel, 8-core SPMD. v3.

Strategy (node-partitioned): core k owns dst nodes [k*6250, (k+1)*6250).
Host sorts each core's edges by (src-table-half, dst-window); windows are
128 dst nodes (== one node tile of the core's slice). Per layer:
 - Node phase: PE transforms the core's node slice (lhsT = x^T tiles,
   rhs = [w_l^T | w_r^T]); x_l slice is written p-major to DRAM and
   AllGathered into the global gather table; x_r stays in SBUF as
   [128, WINS, HID] (window-node on partitions).
 - Edge phase (the hot loop), per 32-tile chunk:
   * x_j = dma_gather(xl_table, idx) -- the ONLY per-edge DMA gather.
     Q7 SWDGE descriptor generation (~8ns/row, 2 Q7 cores, serialized)
     is the machine bottleneck, so v2/v3 eliminated the per-edge x_r
     gather and the rest of the pipeline hides under the gathers.
   * x_rd per edge via PE: psum = S^T.T @ xr_win (S^T [128n,128e] bf16
     from host; zero cols for pad slots). Evacuated 4 tiles per ACT
     copy (batched psum->sbuf).
   * alpha = sum_c leakyrelu(x_j + x_rd, 0.2)*att, w = exp(alpha)
     (softmax max-shift cancels; alphas are small so exp is safe).
   * Segment sum on PE: psum[128n, 130] += S.T @ [w*x_j | w] over each
     (window, part) tile group; groups dumped into an SBUF acc.
 - Finalize: h = acc[:, :, :128]/(denom+1e-16) + bias (+leaky 0.01 for
   layer 1). Layer 2 reuses the identical edge structure and the same
   DRAM table/row mapping. Output = h1 + h2 of the owned slice.
"""
import numpy as np
import ml_dtypes

BF = ml_dtypes.bfloat16

N = 50000
E = 640000
HID = 128
HEADS = 2
C = 64
ATT_SLOPE = 0.2
OUT_SLOPE = 0.01
NCORES = 8
TILE = 128
WIN = 128
SPLIT = 32768
CHUNK_TILES = 24
ATT_TILES = 16
XRD_BATCH = 4


def configure(n=50000, e=640000):
    global N, E, NPC, NPC_PAD, WINS, NTN, NP_GLOB
    N, E = n, e
    NPC = N // NCORES
    NPC_PAD = ((NPC + TILE - 1) // TILE) * TILE
    NTN = NPC_PAD // TILE
    WINS = NPC_PAD // WIN
    NP_GLOB = NPC_PAD * NCORES


configure()


def _pack_idx16(idx):
    n = len(idx)
    cols = -(-n // 16)
    buf = np.zeros((cols, 16), dtype=np.int16)
    buf.reshape(-1)[:n] = idx.astype(np.int16)
    return np.tile(buf.T, (8, 1))


def prep_edges(edge_index):
    src = np.asarray(edge_index[0], dtype=np.int64)
    dst = np.asarray(edge_index[1], dtype=np.int64)
    core_of = dst // NPC
    gp_all = (src // NPC) * NPC_PAD + (src % NPC)
    # p-major row in the gather table: node (k, loc) -> k*NPC_PAD + (loc%128)*NTN + loc//128
    row_all = (gp_all // NPC_PAD) * NPC_PAD + (gp_all % NPC_PAD % 128) * NTN \
        + (gp_all % NPC_PAD) // 128

    cores = []
    cnt = np.zeros((NCORES, WINS, 2), dtype=np.int64)
    for k in range(NCORES):
        m = core_of == k
        rj = row_all[m]
        dl = dst[m] - k * NPC
        order = np.argsort(dl, kind="stable")
        rj, dl = rj[order], dl[order]
        w = dl // WIN
        a = rj < SPLIT
        cnt[k, :, 0] = np.bincount(w[a], minlength=WINS)
        cnt[k, :, 1] = np.bincount(w[~a], minlength=WINS)
        cores.append((rj, dl, w, a))

    budget = (-(-cnt // TILE)).max(axis=0)      # [WINS, 2]
    tile_win, tile_part = [], []
    for part in (0, 1):
        for wi in range(WINS):
            tile_win += [wi] * int(budget[wi, part])
            tile_part += [part] * int(budget[wi, part])
    t_tot = len(tile_win)
    t_a = int(budget[:, 0].sum())

    per_core = []
    for k in range(NCORES):
        rj, dl, w, a = cores[k]
        idx_j = np.zeros(t_tot * TILE, dtype=np.int16)
        dstw = np.full(t_tot * TILE, -1.0, dtype=np.float32)
        pos = 0
        for part in (0, 1):
            mp = a if part == 0 else ~a
            for wi in range(WINS):
                sel = mp & (w == wi)
                r_sel, d_sel = rj[sel], dl[sel]
                n_real = len(r_sel)
                n_slot = int(budget[wi, part]) * TILE
                idx_j[pos:pos + n_real] = (r_sel - (SPLIT if part else 0)).astype(np.int16)
                dstw[pos:pos + n_real] = (d_sel - wi * WIN).astype(np.float32)
                pos += n_slot
        assert pos == t_tot * TILE

        dstw_slots = dstw.reshape(t_tot, TILE)          # [t, e]
        sT = (np.arange(WIN)[:, None, None] ==
              dstw_slots[None, :, :]).astype(BF)        # [n, t, e]
        s_mat = sT.transpose(2, 1, 0)                   # [e, t, n]
        per_core.append({
            "idxJ": _pack_idx16(idx_j),
            "smat": np.ascontiguousarray(s_mat),
            "smatT": np.ascontiguousarray(sT),
        })

    meta = {"t_tot": t_tot, "t_a": t_a,
            "tile_win": tile_win, "tile_part": tile_part}
    return meta, per_core


def build_bass(meta):
    from concourse import bacc, mybir, tile

    F32, BF16, I16 = mybir.dt.float32, mybir.dt.bfloat16, mybir.dt.int16
    AF = mybir.ActivationFunctionType
    ALU = mybir.AluOpType

    t_tot, t_a = meta["t_tot"], meta["t_a"]
    tile_win, tile_part = meta["tile_win"], meta["tile_part"]
    n_chunks = -(-t_tot // CHUNK_TILES)

    nc = bacc.Bacc("TRN2", target_bir_lowering=False, debug=False,
                   num_devices=NCORES)

    xT_d = nc.dram_tensor("xT", [HID, NPC_PAD], BF16, kind="ExternalInput")
    w1_d = nc.dram_tensor("w1T", [HID, 2 * HID], BF16, kind="ExternalInput")
    w2_d = nc.dram_tensor("w2T", [HID, 2 * HID], BF16, kind="ExternalInput")
    att1_d = nc.dram_tensor("attbc1", [128, ATT_TILES * HID], BF16,
                            kind="ExternalInput")
    att2_d = nc.dram_tensor("attbc2", [128, ATT_TILES * HID], BF16,
                            kind="ExternalInput")
    b1_d = nc.dram_tensor("bias1", [128, HID], F32, kind="ExternalInput")
    b2_d = nc.dram_tensor("bias2", [128, HID], F32, kind="ExternalInput")
    id_d = nc.dram_tensor("ident", [128, 128], BF16, kind="ExternalInput")
    idxj_d = nc.dram_tensor("idxJ", [128, t_tot * 8], I16, kind="ExternalInput")
    smat_d = nc.dram_tensor("smat", [128, t_tot, WIN], BF16, kind="ExternalInput")
    smatT_d = nc.dram_tensor("smatT", [WIN, t_tot, TILE], BF16, kind="ExternalInput")
    out_d = nc.dram_tensor("out", [128, WINS, HID], BF16, kind="ExternalOutput")

    with tile.TileContext(nc) as tc:
        with (
            tc.tile_pool(name="const", bufs=1) as cpool,
            tc.tile_pool(name="node", bufs=1) as npool,
            tc.tile_pool(name="nstage", bufs=1) as spool,
            tc.tile_pool(name="gat", bufs=3) as gpool,
            tc.tile_pool(name="edge", bufs=2) as epool,
            tc.tile_pool(name="mmps", bufs=2, space="PSUM") as mmps,
            tc.tile_pool(name="wps", bufs=2, space="PSUM") as wps,
            tc.tile_pool(name="dram", bufs=1, space="DRAM") as dpool,
        ):
            w1_t = cpool.tile([HID, 2 * HID], BF16, tag="w1")
            w2_t = cpool.tile([HID, 2 * HID], BF16, tag="w2")
            att1_t = cpool.tile([128, ATT_TILES, HID], BF16, tag="att1")
            att2_t = cpool.tile([128, ATT_TILES, HID], BF16, tag="att2")
            b1_t = cpool.tile([128, HID], F32, tag="b1")
            b2_t = cpool.tile([128, HID], F32, tag="b2")
            id_t = cpool.tile([128, 128], BF16, tag="ident")
            idxj_t = cpool.tile([128, t_tot * 8], I16, tag="idxj")

            for tdst, tsrc in ((w1_t, w1_d), (w2_t, w2_d),
                               (att1_t.rearrange("p t f -> p (t f)"), att1_d),
                               (att2_t.rearrange("p t f -> p (t f)"), att2_d),
                               (b1_t, b1_d), (b2_t, b2_d),
                               (id_t, id_d), (idxj_t, idxj_d)):
                nc.sync.dma_start(tdst[:], tsrc[:])

            xT_t = npool.tile([HID, NPC_PAD], BF16, tag="xT")
            nc.sync.dma_start(xT_t[:], xT_d[:])

            h1_t = npool.tile([128, WINS, HID], BF16, tag="h1")
            h1T_t = xT_t  # xT is dead after layer-1 node phase; reuse the slot
            acc_t = npool.tile([128, WINS, HID + 2], F32, tag="acc")
            xr_t = npool.tile([128, WINS, HID], BF16, tag="xr")

            cin = dpool.tile([128, NPC_PAD], BF16, tag="cin")
            xl_fulls = [dpool.tile([NP_GLOB, HID], BF16, tag=f"xlfull{i}",
                                   name=f"xlfull{i}", addr_space="Shared")
                        for i in (0, 1)]

            def leaky(dst_ap, src_ap, slope, tag, shape, dt_, pool, nt=None):
                tmp = pool.tile(shape, dt_, tag=tag)
                tv = tmp[:, :nt, :] if nt is not None else tmp[:]
                nc.scalar.mul(tv, src_ap, slope)
                nc.vector.tensor_tensor(out=dst_ap, in0=src_ap,
                                        in1=tv, op=ALU.max)

            def layer(li, lhsT_tile, w_t, att_t, b_t, h_out, apply_leaky):
                xl_full = xl_fulls[li]
                # ---- node phase: xl slice -> DRAM table, xr -> SBUF ----
                xl_sb = spool.tile([128, NTN, HID], BF16, tag="xlsb")
                for t in range(NTN):
                    ps = mmps.tile([128, 2 * HID], F32, tag="nodeps")
                    nc.tensor.matmul(ps[:], lhsT_tile[:, t * 128:(t + 1) * 128],
                                     w_t[:], start=True, stop=True)
                    nc.scalar.copy(xl_sb[:, t, :], ps[:, :HID])
                    nc.vector.tensor_copy(xr_t[:, t, :], ps[:, HID:])
                nc.sync.dma_start(cin[:], xl_sb[:].rearrange("p t f -> p (t f)"))
                nc.gpsimd.collective_compute(
                    "AllGather", mybir.AluOpType.bypass,
                    replica_groups=[list(range(NCORES))],
                    ins=[cin.opt()], outs=[xl_full.opt()])

                nc.vector.memset(acc_t[:], 0.0)

                # ---- edge phase ----
                cur_ps = None
                for ci in range(n_chunks):
                    t0 = ci * CHUNK_TILES
                    t1 = min(t0 + CHUNK_TILES, t_tot)
                    nt = t1 - t0

                    xj = gpool.tile([128, CHUNK_TILES, HID], BF16, tag="xj")
                    spans = []
                    if t0 < t_a:
                        spans.append((t0, min(t1, t_a), 0))
                    if t1 > t_a:
                        spans.append((max(t0, t_a), t1, 1))
                    for (sa, sb_, part) in spans:
                        n_i = (sb_ - sa) * TILE
                        tab = xl_full[SPLIT:NP_GLOB, :] if part else \
                            xl_full[0:SPLIT, :]
                        nc.gpsimd.dma_gather(
                            out_ap=xj[:, sa - t0:sb_ - t0, :], in_ap=tab,
                            idxs_ap=idxj_t[:, sa * 8:sb_ * 8],
                            num_idxs=n_i, num_idxs_reg=n_i, elem_size=HID,
                            single_packet=False)

                    sT_t = epool.tile([WIN, CHUNK_TILES, TILE], BF16, tag="smt")
                    nc.sync.dma_start(sT_t[:, :nt, :], smatT_d[:, t0:t1, :])
                    s_t = epool.tile([128, CHUNK_TILES, WIN], BF16, tag="smat")
                    nc.sync.dma_start(s_t[:, :nt, :], smat_d[:, t0:t1, :])

                    # x_rd per edge via PE; batched psum->sbuf evacuation
                    xrd = epool.tile([128, CHUNK_TILES, HID], BF16, tag="xrd")
                    for tb in range(t0, t1, XRD_BATCH):
                        te = min(tb + XRD_BATCH, t1)
                        xps = mmps.tile([128, XRD_BATCH, HID], F32, tag="xrdps")
                        for t in range(tb, te):
                            nc.tensor.matmul(xps[:, t - tb, :],
                                             sT_t[:, t - t0, :],
                                             xr_t[:, tile_win[t], :],
                                             start=True, stop=True)
                        nc.scalar.copy(xrd[:, tb - t0:te - t0, :],
                                       xps[:, :te - tb, :])

                    ev = epool.tile([128, CHUNK_TILES, HID], BF16, tag="ev")
                    nc.vector.tensor_tensor(out=ev[:, :nt, :], in0=xj[:, :nt, :],
                                            in1=xrd[:, :nt, :], op=ALU.add)
                    leaky(ev[:, :nt, :], ev[:, :nt, :], ATT_SLOPE, "lk",
                          [128, CHUNK_TILES, HID], BF16, epool, nt=nt)

                    alph = epool.tile([128, CHUNK_TILES, 2], F32, tag="alph")
                    prod = epool.tile([128, CHUNK_TILES, HID], BF16, tag="lk")
                    for j0 in range(0, nt, ATT_TILES):
                        je = min(j0 + ATT_TILES, nt)
                        nc.vector.tensor_tensor(
                            out=prod[:, j0:je, :], in0=ev[:, j0:je, :],
                            in1=att_t[:, :je - j0, :], op=ALU.mult)
                    nc.vector.tensor_reduce(
                        out=alph[:, :nt, :],
                        in_=prod[:, :nt, :].rearrange("p t (h c) -> p t h c", h=2),
                        axis=mybir.AxisListType.X, op=ALU.add)
                    rhs = epool.tile([128, CHUNK_TILES, HID + 2], BF16, tag="rhs")
                    nc.scalar.activation(rhs[:, :nt, HID:], alph[:, :nt, :], AF.Exp)
                    nc.vector.tensor_tensor(
                        out=rhs[:, :nt, :HID].rearrange("p t (h c) -> p t h c", h=2),
                        in0=xj[:, :nt, :].rearrange("p t (h c) -> p t h c", h=2),
                        in1=rhs[:, :nt, HID:].unsqueeze(3).to_broadcast(
                            [128, nt, 2, C]),
                        op=ALU.mult)

                    for t in range(t0, t1):
                        wi, part = tile_win[t], tile_part[t]
                        if cur_ps is None:
                            cur_ps = wps.tile([WIN, HID + 2], F32, tag="winps",
                                              name=f"winps_{li}_{wi}_{part}")
                        first = (t == 0) or (tile_win[t - 1], tile_part[t - 1]) != (wi, part)
                        last = (t == t_tot - 1) or \
                            (tile_win[t + 1], tile_part[t + 1]) != (wi, part)
                        nc.tensor.matmul(cur_ps[:], s_t[:, t - t0, :],
                                         rhs[:, t - t0, :],
                                         start=first, stop=last)
                        if last:
                            dst = acc_t[:, wi, :HID + 2]
                            nc.vector.tensor_tensor(out=dst, in0=dst,
                                                    in1=cur_ps[:], op=ALU.add)
                            cur_ps = None

                # ---- finalize ----
                den = npool.tile([128, WINS, 2], F32, tag="den")
                nc.vector.tensor_scalar_add(den[:], acc_t[:, :, HID:], 1e-16)
                rden = npool.tile([128, WINS, 2], F32, tag="rden")
                nc.vector.reciprocal(rden[:], den[:])
                nc.vector.tensor_tensor(
                    out=h_out[:].rearrange("p t (h c) -> p t h c", h=2),
                    in0=acc_t[:, :, :HID].rearrange("p t (h c) -> p t h c", h=2),
                    in1=rden[:].unsqueeze(3).to_broadcast([128, WINS, 2, C]),
                    op=ALU.mult)
                nc.vector.tensor_tensor(
                    out=h_out[:], in0=h_out[:],
                    in1=b_t[:].unsqueeze(1).to_broadcast([128, WINS, HID]),
                    op=ALU.add)
                if apply_leaky:
                    leaky(h_out[:], h_out[:], OUT_SLOPE, "xlsb",
                          [128, NTN, HID], BF16, spool)

            # ---------------- layer 1 ----------------
            layer(0, xT_t, w1_t, att1_t, b1_t, h1_t, apply_leaky=True)

            # h1^T tiles for layer 2 (PE transpose)
            for t in range(NTN):
                tp = mmps.tile([128, 128], BF16, tag="tps")
                nc.tensor.transpose(tp[:], h1_t[:, t, :], id_t[:])
                nc.scalar.copy(h1T_t[:, t * 128:(t + 1) * 128], tp[:])

            # ---------------- layer 2 ----------------
            h2_t = npool.tile([128, WINS, HID], BF16, tag="h2")
            layer(1, h1T_t, w2_t, att2_t, b2_t, h2_t, apply_leaky=False)

            nc.vector.tensor_tensor(out=h2_t[:], in0=h2_t[:], in1=h1_t[:],
                                    op=ALU.add)
            nc.sync.dma_start(out_d[:], h2_t[:])

    nc.compile()
    return nc


def make_inputs(x, edge_index, w_l1, w_r1, att1, b1, w_l2, w_r2, att2, b2):
    """Host-side prep: returns (meta, in_maps)."""
    meta, per_core = prep_edges(edge_index)
    x = np.asarray(x, dtype=np.float32)
    ident = np.eye(128, dtype=np.float32).astype(BF)

    def wcat(wl, wr):
        return np.concatenate([np.asarray(wl).T, np.asarray(wr).T],
                              axis=1).astype(BF)

    def att_tiled(a):
        row = np.asarray(a).reshape(1, HID)
        return np.tile(row, (128, ATT_TILES)).astype(BF)

    b_bc = lambda b: np.tile(np.asarray(b).reshape(1, HID),
                             (128, 1)).astype(np.float32)

    w1 = wcat(w_l1, w_r1)
    w2 = wcat(w_l2, w_r2)
    a1, a2 = att_tiled(att1), att_tiled(att2)
    bb1, bb2 = b_bc(b1), b_bc(b2)

    in_maps = []
    for k in range(NCORES):
        xs = np.zeros((NPC_PAD, HID), dtype=np.float32)
        xs[:NPC] = x[k * NPC:(k + 1) * NPC]
        in_maps.append({
            "xT": np.ascontiguousarray(xs.T).astype(BF),
            "w1T": w1, "w2T": w2, "attbc1": a1, "attbc2": a2,
            "bias1": bb1, "bias2": bb2, "ident": ident,
            **per_core[k],
        })
    return meta, in_maps


def kernel(**inputs):
    from concourse.bass_utils import run_bass_kernel_spmd

    meta, in_maps = make_inputs(**inputs)
    nc = build_bass(meta)
    res = run_bass_kernel_spmd(nc, in_maps, list(range(NCORES)))
    outs = []
    for k in range(NCORES):
        o = res.results[k]["out"]          # [128, WINS, HID]
        outs.append(o.transpose(1, 0, 2).reshape(NPC_PAD, HID)[:NPC])
    return np.concatenate(outs, axis=0).astype(np.float32)


# revision 15
# speedup vs baseline: 2.6805x; 1.0347x over previous
"""GATv2 (2-layer) Trainium2 Bass kernel, 8-core SPMD. v3.

Strategy (node-partitioned): core k owns dst nodes [k*6250, (k+1)*6250).
Host sorts each core's edges by (src-table-half, dst-window); windows are
128 dst nodes (== one node tile of the core's slice). Per layer:
 - Node phase: PE transforms the core's node slice (lhsT = x^T tiles,
   rhs = [w_l^T | w_r^T]); x_l slice is written p-major to DRAM and
   AllGathered into the global gather table; x_r stays in SBUF as
   [128, WINS, HID] (window-node on partitions).
 - Edge phase (the hot loop), per 32-tile chunk:
   * x_j = dma_gather(xl_table, idx) -- the ONLY per-edge DMA gather.
     Q7 SWDGE descriptor generation (~8ns/row, 2 Q7 cores, serialized)
     is the machine bottleneck, so v2/v3 eliminated the per-edge x_r
     gather and the rest of the pipeline hides under the gathers.
   * x_rd per edge via PE: psum = S^T.T @ xr_win (S^T [128n,128e] bf16
     from host; zero cols for pad slots). Evacuated 4 tiles per ACT
     copy (batched psum->sbuf).
   * alpha = sum_c leakyrelu(x_j + x_rd, 0.2)*att, w = exp(alpha)
     (softmax max-shift cancels; alphas are small so exp is safe).
   * Segment sum on PE: psum[128n, 130] += S.T @ [w*x_j | w] over each
     (window, part) tile group; groups dumped into an SBUF acc.
 - Finalize: h = acc[:, :, :128]/(denom+1e-16) + bias (+leaky 0.01 for
   layer 1). Layer 2 reuses the identical edge structure and the same
   DRAM table/row mapping. Output = h1 + h2 of the owned slice.
"""
import numpy as np
import ml_dtypes

BF = ml_dtypes.bfloat16

N = 50000
E = 640000
HID = 128
HEADS = 2
C = 64
ATT_SLOPE = 0.2
OUT_SLOPE = 0.01
NCORES = 8
TILE = 128
WIN = 128
SPLIT = 32768
CHUNK_TILES = 24
ATT_TILES = 16
XRD_BATCH = 4
WBATCH = 10


def configure(n=50000, e=640000):
    global N, E, NPC, NPC_PAD, WINS, NTN, NP_GLOB
    N, E = n, e
    NPC = N // NCORES
    NPC_PAD = ((NPC + TILE - 1) // TILE) * TILE
    NTN = NPC_PAD // TILE
    WINS = NPC_PAD // WIN
    NP_GLOB = NPC_PAD * NCORES


configure()


def _pack_idx16(idx):
    n = len(idx)
    cols = -(-n // 16)
    buf = np.zeros((cols, 16), dtype=np.int16)
    buf.reshape(-1)[:n] = idx.astype(np.int16)
    return np.tile(buf.T, (8, 1))


def prep_edges(edge_index):
    src = np.asarray(edge_index[0], dtype=np.int64)
    dst = np.asarray(edge_index[1], dtype=np.int64)
    core_of = dst // NPC
    gp_all = (src // NPC) * NPC_PAD + (src % NPC)
    # p-major row in the gather table: node (k, loc) -> k*NPC_PAD + (loc%128)*NTN + loc//128
    row_all = (gp_all // NPC_PAD) * NPC_PAD + (gp_all % NPC_PAD % 128) * NTN \
        + (gp_all % NPC_PAD) // 128

    cores = []
    cnt = np.zeros((NCORES, WINS, 2), dtype=np.int64)
    for k in range(NCORES):
        m = core_of == k
        rj = row_all[m]
        dl = dst[m] - k * NPC
        order = np.argsort(dl, kind="stable")
        rj, dl = rj[order], dl[order]
        w = dl // WIN
        a = rj < SPLIT
        cnt[k, :, 0] = np.bincount(w[a], minlength=WINS)
        cnt[k, :, 1] = np.bincount(w[~a], minlength=WINS)
        cores.append((rj, dl, w, a))

    budget = (-(-cnt // TILE)).max(axis=0)      # [WINS, 2]
    tile_win, tile_part = [], []
    for part in (0, 1):
        for wi in range(WINS):
            tile_win += [wi] * int(budget[wi, part])
            tile_part += [part] * int(budget[wi, part])
    t_tot = len(tile_win)
    t_a = int(budget[:, 0].sum())

    per_core = []
    for k in range(NCORES):
        rj, dl, w, a = cores[k]
        idx_j = np.zeros(t_tot * TILE, dtype=np.int16)
        dstw = np.full(t_tot * TILE, -1.0, dtype=np.float32)
        pos = 0
        for part in (0, 1):
            mp = a if part == 0 else ~a
            for wi in range(WINS):
                sel = mp & (w == wi)
                r_sel, d_sel = rj[sel], dl[sel]
                n_real = len(r_sel)
                n_slot = int(budget[wi, part]) * TILE
                idx_j[pos:pos + n_real] = (r_sel - (SPLIT if part else 0)).astype(np.int16)
                dstw[pos:pos + n_real] = (d_sel - wi * WIN).astype(np.float32)
                pos += n_slot
        assert pos == t_tot * TILE

        dstw_slots = dstw.reshape(t_tot, TILE)          # [t, e]
        sT = (np.arange(WIN)[:, None, None] ==
              dstw_slots[None, :, :]).astype(BF)        # [n, t, e]
        s_mat = sT.transpose(2, 1, 0)                   # [e, t, n]
        per_core.append({
            "idxJ": _pack_idx16(idx_j),
            "smat": np.ascontiguousarray(s_mat),
            "smatT": np.ascontiguousarray(sT),
        })

    meta = {"t_tot": t_tot, "t_a": t_a,
            "tile_win": tile_win, "tile_part": tile_part}
    return meta, per_core


def build_bass(meta):
    from concourse import bacc, mybir, tile

    F32, BF16, I16 = mybir.dt.float32, mybir.dt.bfloat16, mybir.dt.int16
    AF = mybir.ActivationFunctionType
    ALU = mybir.AluOpType

    t_tot, t_a = meta["t_tot"], meta["t_a"]
    tile_win, tile_part = meta["tile_win"], meta["tile_part"]
    n_chunks = -(-t_tot // CHUNK_TILES)
    n_batches = -(-WINS // WBATCH)
    last_tile_of_batch = {}
    for b in range(n_batches):
        wset = set(range(b * WBATCH, min((b + 1) * WBATCH, WINS)))
        lt = max(t for t in range(t_tot) if tile_win[t] in wset)
        last_tile_of_batch[lt] = b

    nc = bacc.Bacc("TRN2", target_bir_lowering=False, debug=False,
                   num_devices=NCORES)

    xT_d = nc.dram_tensor("xT", [HID, NPC_PAD], BF16, kind="ExternalInput")
    w1_d = nc.dram_tensor("w1T", [HID, 2 * HID], BF16, kind="ExternalInput")
    w2_d = nc.dram_tensor("w2T", [HID, 2 * HID], BF16, kind="ExternalInput")
    att1_d = nc.dram_tensor("attbc1", [128, ATT_TILES * HID], BF16,
                            kind="ExternalInput")
    att2_d = nc.dram_tensor("attbc2", [128, ATT_TILES * HID], BF16,
                            kind="ExternalInput")
    b1_d = nc.dram_tensor("bias1", [128, HID], F32, kind="ExternalInput")
    b2_d = nc.dram_tensor("bias2", [128, HID], F32, kind="ExternalInput")
    id_d = nc.dram_tensor("ident", [128, 128], BF16, kind="ExternalInput")
    idxj_d = nc.dram_tensor("idxJ", [128, t_tot * 8], I16, kind="ExternalInput")
    smat_d = nc.dram_tensor("smat", [128, t_tot, WIN], BF16, kind="ExternalInput")
    smatT_d = nc.dram_tensor("smatT", [WIN, t_tot, TILE], BF16, kind="ExternalInput")
    out_d = nc.dram_tensor("out", [128, WINS, HID], BF16, kind="ExternalOutput")

    with tile.TileContext(nc) as tc:
        with (
            tc.tile_pool(name="const", bufs=1) as cpool,
            tc.tile_pool(name="node", bufs=1) as npool,
            tc.tile_pool(name="gat", bufs=3) as gpool,
            tc.tile_pool(name="edge", bufs=2) as epool,
            tc.tile_pool(name="mmps", bufs=2, space="PSUM") as mmps,
            tc.tile_pool(name="wps", bufs=2, space="PSUM") as wps,
            tc.tile_pool(name="dram", bufs=1, space="DRAM") as dpool,
        ):
            w1_t = cpool.tile([HID, 2 * HID], BF16, tag="w1")
            w2_t = cpool.tile([HID, 2 * HID], BF16, tag="w2")
            att1_t = cpool.tile([128, ATT_TILES, HID], BF16, tag="att1")
            att2_t = cpool.tile([128, ATT_TILES, HID], BF16, tag="att2")
            b1_t = cpool.tile([128, HID], F32, tag="b1")
            b2_t = cpool.tile([128, HID], F32, tag="b2")
            id_t = cpool.tile([128, 128], BF16, tag="ident")
            idxj_t = cpool.tile([128, t_tot * 8], I16, tag="idxj")

            for tdst, tsrc in ((w1_t, w1_d), (w2_t, w2_d),
                               (att1_t.rearrange("p t f -> p (t f)"), att1_d),
                               (att2_t.rearrange("p t f -> p (t f)"), att2_d),
                               (b1_t, b1_d), (b2_t, b2_d),
                               (id_t, id_d), (idxj_t, idxj_d)):
                nc.sync.dma_start(tdst[:], tsrc[:])

            xT_t = npool.tile([HID, NPC_PAD], BF16, tag="xT")
            nc.sync.dma_start(xT_t[:], xT_d[:])

            h1_t = npool.tile([128, WINS, HID], BF16, tag="h1")
            h1T_t = xT_t  # xT is dead after layer-1 node phase; reuse the slot
            acc_t = npool.tile([128, WINS, HID + 2], F32, tag="acc")
            xr_t = npool.tile([128, WINS, HID], BF16, tag="xr")
            xl_sb = npool.tile([128, NTN, HID], BF16, tag="xlsb")

            cin = dpool.tile([128, NPC_PAD], BF16, tag="cin")
            xl_fulls = [dpool.tile([NP_GLOB, HID], BF16, tag=f"xlfull{i}",
                                   name=f"xlfull{i}", addr_space="Shared")
                        for i in (0, 1)]

            def leaky(dst_ap, src_ap, slope, tag, shape, dt_, pool, nt=None):
                tmp = pool.tile(shape, dt_, tag=tag)
                tv = tmp[:, :nt, :] if nt is not None else tmp[:]
                nc.scalar.mul(tv, src_ap, slope)
                nc.vector.tensor_tensor(out=dst_ap, in0=src_ap,
                                        in1=tv, op=ALU.max)

            def layer(li, lhsT_tile, w_t, att_t, b_t, on_batch):
                xl_full = xl_fulls[li]
                # ---- node phase (layer 1 only; layer 2's transform is
                # emitted per-batch from layer 1's on_batch callback) ----
                if li == 0:
                    for t in range(NTN):
                        ps = mmps.tile([128, 2 * HID], F32, tag="nodeps")
                        nc.tensor.matmul(ps[:], lhsT_tile[:, t * 128:(t + 1) * 128],
                                         w_t[:], start=True, stop=True)
                        nc.scalar.copy(xl_sb[:, t, :], ps[:, :HID])
                        nc.vector.tensor_copy(xr_t[:, t, :], ps[:, HID:])
                nc.sync.dma_start(cin[:], xl_sb[:].rearrange("p t f -> p (t f)"))
                nc.gpsimd.collective_compute(
                    "AllGather", mybir.AluOpType.bypass,
                    replica_groups=[list(range(NCORES))],
                    ins=[cin.opt()], outs=[xl_full.opt()])

                nc.vector.memset(acc_t[:], 0.0)

                # ---- edge phase ----
                cur_ps = None
                for ci in range(n_chunks):
                    t0 = ci * CHUNK_TILES
                    t1 = min(t0 + CHUNK_TILES, t_tot)
                    nt = t1 - t0

                    xj = gpool.tile([128, CHUNK_TILES, HID], BF16, tag="xj")
                    spans = []
                    if t0 < t_a:
                        spans.append((t0, min(t1, t_a), 0))
                    if t1 > t_a:
                        spans.append((max(t0, t_a), t1, 1))
                    for (sa, sb_, part) in spans:
                        n_i = (sb_ - sa) * TILE
                        tab = xl_full[SPLIT:NP_GLOB, :] if part else \
                            xl_full[0:SPLIT, :]
                        nc.gpsimd.dma_gather(
                            out_ap=xj[:, sa - t0:sb_ - t0, :], in_ap=tab,
                            idxs_ap=idxj_t[:, sa * 8:sb_ * 8],
                            num_idxs=n_i, num_idxs_reg=n_i, elem_size=HID,
                            single_packet=False)

                    sT_t = epool.tile([WIN, CHUNK_TILES, TILE], BF16, tag="smt")
                    nc.sync.dma_start(sT_t[:, :nt, :], smatT_d[:, t0:t1, :])
                    s_t = epool.tile([128, CHUNK_TILES, WIN], BF16, tag="smat")
                    nc.sync.dma_start(s_t[:, :nt, :], smat_d[:, t0:t1, :])

                    # x_rd per edge via PE; batched psum->sbuf evacuation
                    xrd = epool.tile([128, CHUNK_TILES, HID], BF16, tag="xrd")
                    for tb in range(t0, t1, XRD_BATCH):
                        te = min(tb + XRD_BATCH, t1)
                        xps = mmps.tile([128, XRD_BATCH, HID], F32, tag="xrdps")
                        for t in range(tb, te):
                            nc.tensor.matmul(xps[:, t - tb, :],
                                             sT_t[:, t - t0, :],
                                             xr_t[:, tile_win[t], :],
                                             start=True, stop=True)
                        nc.scalar.copy(xrd[:, tb - t0:te - t0, :],
                                       xps[:, :te - tb, :])

                    ev = epool.tile([128, CHUNK_TILES, HID], BF16, tag="ev")
                    nc.vector.tensor_tensor(out=ev[:, :nt, :], in0=xj[:, :nt, :],
                                            in1=xrd[:, :nt, :], op=ALU.add)
                    leaky(ev[:, :nt, :], ev[:, :nt, :], ATT_SLOPE, "lk",
                          [128, CHUNK_TILES, HID], BF16, epool, nt=nt)

                    alph = epool.tile([128, CHUNK_TILES, 2], F32, tag="alph")
                    prod = epool.tile([128, CHUNK_TILES, HID], BF16, tag="lk")
                    for j0 in range(0, nt, ATT_TILES):
                        je = min(j0 + ATT_TILES, nt)
                        nc.vector.tensor_tensor(
                            out=prod[:, j0:je, :], in0=ev[:, j0:je, :],
                            in1=att_t[:, :je - j0, :], op=ALU.mult)
                    nc.vector.tensor_reduce(
                        out=alph[:, :nt, :],
                        in_=prod[:, :nt, :].rearrange("p t (h c) -> p t h c", h=2),
                        axis=mybir.AxisListType.X, op=ALU.add)
                    rhs = epool.tile([128, CHUNK_TILES, HID + 2], BF16, tag="rhs")
                    nc.scalar.activation(rhs[:, :nt, HID:], alph[:, :nt, :], AF.Exp)
                    nc.vector.tensor_tensor(
                        out=rhs[:, :nt, :HID].rearrange("p t (h c) -> p t h c", h=2),
                        in0=xj[:, :nt, :].rearrange("p t (h c) -> p t h c", h=2),
                        in1=rhs[:, :nt, HID:].unsqueeze(3).to_broadcast(
                            [128, nt, 2, C]),
                        op=ALU.mult)

                    for t in range(t0, t1):
                        wi, part = tile_win[t], tile_part[t]
                        if cur_ps is None:
                            cur_ps = wps.tile([WIN, HID + 2], F32, tag="winps",
                                              name=f"winps_{li}_{wi}_{part}")
                        first = (t == 0) or (tile_win[t - 1], tile_part[t - 1]) != (wi, part)
                        last = (t == t_tot - 1) or \
                            (tile_win[t + 1], tile_part[t + 1]) != (wi, part)
                        nc.tensor.matmul(cur_ps[:], s_t[:, t - t0, :],
                                         rhs[:, t - t0, :],
                                         start=first, stop=last)
                        if last:
                            dst = acc_t[:, wi, :HID + 2]
                            nc.vector.tensor_tensor(out=dst, in0=dst,
                                                    in1=cur_ps[:], op=ALU.add)
                            cur_ps = None
                            if t in last_tile_of_batch:
                                b = last_tile_of_batch[t]
                                on_batch(b * WBATCH,
                                         min((b + 1) * WBATCH, WINS))

            def finalize_batch(h_out, b_t, w0, w1, apply_leaky):
                nw = w1 - w0
                den = epool.tile([128, WBATCH, 2], F32, tag="den")
                nc.vector.tensor_scalar_add(den[:, :nw, :],
                                            acc_t[:, w0:w1, HID:], 1e-16)
                rden = epool.tile([128, WBATCH, 2], F32, tag="rden")
                nc.vector.reciprocal(rden[:, :nw, :], den[:, :nw, :])
                nc.vector.tensor_tensor(
                    out=h_out[:, w0:w1, :].rearrange("p t (h c) -> p t h c", h=2),
                    in0=acc_t[:, w0:w1, :HID].rearrange("p t (h c) -> p t h c", h=2),
                    in1=rden[:, :nw, :].unsqueeze(3).to_broadcast(
                        [128, nw, 2, C]),
                    op=ALU.mult)
                nc.vector.tensor_tensor(
                    out=h_out[:, w0:w1, :], in0=h_out[:, w0:w1, :],
                    in1=b_t[:].unsqueeze(1).to_broadcast([128, nw, HID]),
                    op=ALU.add)
                if apply_leaky:
                    leaky(h_out[:, w0:w1, :], h_out[:, w0:w1, :], OUT_SLOPE,
                          "lkb", [128, WBATCH, HID], BF16, epool, nt=nw)

            h2_t = npool.tile([128, WINS, HID], BF16, tag="h2")

            def l1_batch(w0, w1):
                finalize_batch(h1_t, b1_t, w0, w1, apply_leaky=True)
                for t in range(w0, w1):
                    tp = mmps.tile([128, 128], BF16, tag="tps")
                    nc.tensor.transpose(tp[:], h1_t[:, t, :], id_t[:])
                    nc.scalar.copy(h1T_t[:, t * 128:(t + 1) * 128], tp[:])
                    ps = mmps.tile([128, 2 * HID], F32, tag="nodeps")
                    nc.tensor.matmul(ps[:], h1T_t[:, t * 128:(t + 1) * 128],
                                     w2_t[:], start=True, stop=True)
                    nc.scalar.copy(xl_sb[:, t, :], ps[:, :HID])
                    nc.vector.tensor_copy(xr_t[:, t, :], ps[:, HID:])

            def l2_batch(w0, w1):
                finalize_batch(h2_t, b2_t, w0, w1, apply_leaky=False)
                nc.vector.tensor_tensor(out=h2_t[:, w0:w1, :],
                                        in0=h2_t[:, w0:w1, :],
                                        in1=h1_t[:, w0:w1, :], op=ALU.add)
                nc.sync.dma_start(out_d[:, w0:w1, :], h2_t[:, w0:w1, :])

            # ---------------- layer 1 ----------------
            layer(0, xT_t, w1_t, att1_t, b1_t, l1_batch)
            # ---------------- layer 2 ----------------
            layer(1, h1T_t, w2_t, att2_t, b2_t, l2_batch)

    nc.compile()
    return nc


def make_inputs(x, edge_index, w_l1, w_r1, att1, b1, w_l2, w_r2, att2, b2):
    """Host-side prep: returns (meta, in_maps)."""
    meta, per_core = prep_edges(edge_index)
    x = np.asarray(x, dtype=np.float32)
    ident = np.eye(128, dtype=np.float32).astype(BF)

    def wcat(wl, wr):
        return np.concatenate([np.asarray(wl).T, np.asarray(wr).T],
                              axis=1).astype(BF)

    def att_tiled(a):
        row = np.asarray(a).reshape(1, HID)
        return np.tile(row, (128, ATT_TILES)).astype(BF)

    b_bc = lambda b: np.tile(np.asarray(b).reshape(1, HID),
                             (128, 1)).astype(np.float32)

    w1 = wcat(w_l1, w_r1)
    w2 = wcat(w_l2, w_r2)
    a1, a2 = att_tiled(att1), att_tiled(att2)
    bb1, bb2 = b_bc(b1), b_bc(b2)

    in_maps = []
    for k in range(NCORES):
        xs = np.zeros((NPC_PAD, HID), dtype=np.float32)
        xs[:NPC] = x[k * NPC:(k + 1) * NPC]
        in_maps.append({
            "xT": np.ascontiguousarray(xs.T).astype(BF),
            "w1T": w1, "w2T": w2, "attbc1": a1, "attbc2": a2,
            "bias1": bb1, "bias2": bb2, "ident": ident,
            **per_core[k],
        })
    return meta, in_maps


def kernel(**inputs):
    from concourse.bass_utils import run_bass_kernel_spmd

    meta, in_maps = make_inputs(**inputs)
    nc = build_bass(meta)
    res = run_bass_kernel_spmd(nc, in_maps, list(range(NCORES)))
    outs = []
    for k in range(NCORES):
        o = res.results[k]["out"]          # [128, WINS, HID]
        outs.append(o.transpose(1, 0, 2).reshape(NPC_PAD, HID)[:NPC])
    return np.concatenate(outs, axis=0).astype(np.float32)
